# revision 8
# baseline (speedup 1.0000x reference)
"""Trainium2 Bass kernel for CrossAttention (B=8, N=M=2048, C=1024), fp32 io.

Sharding: data-parallel — one batch element per NeuronCore (8 cores).

Per-core pipeline (batch b), all matmuls bf16 (1 cyc/row) or fp8-e4m3
DoubleRow 3-term hi/lo (0.75x bf16 cost), fp32 PSUM accumulation:

  kT[d, m] = (support_perm @ wk^T)^T      fp8 DoubleRow, host-prepped operands
  v[m, d]  = mask_perm/32 * (support_perm @ wv^T)   (only mask=1 m-tiles)
  per n-chunk:
    qT[d, n] = (x @ wq^T)^T               fp8 DoubleRow
    sT[m, n] = kT^T qT   (psum, per m-tile; no transposes anywhere)
    pT = exp(SCALE' * sT) -> bf16         (no max-sub: logits <= ~16 in fp32)
    o[n, d] = sum over mask=1 m-tiles of pT^T @ v      (psum)
    den[n]  = ones-matmul column-sum of pT over ALL m-tiles (free-dim-1
              matmuls are ~free); o_sb = o * 1/den  -> bf16
  per n-half: proj with the swapaxes/reshape fold (contraction over o rows),
              bias add, DMA out.

support rows are permuted on host so mask=1 rows come first: the post-softmax
column mask makes masked-out columns contribute only to the softmax
denominator, so p@v and the v build skip them entirely (exact, not approx).
Weights are scaled x32 on host so fp8 hi/lo residuals stay in e4m3 normal
range; the 1/32 factors fold into the exp scale and the mask multiplier.
"""

import sys

sys.path.insert(0, "/opt/trn_rl_repo")

import numpy as np

import concourse.bass as bass
import concourse.tile as tile
from concourse import bacc, mybir
from concourse.bass_utils import run_bass_kernel_spmd

F32 = mybir.dt.float32
BF16 = mybir.dt.bfloat16
F8 = mybir.dt.float8e4
AF = mybir.ActivationFunctionType
PM = mybir.MatmulPerfMode.DoubleRow
F8NP = mybir.dt.np(F8)
BFNP = mybir.dt.np(BF16)

B, N, M, C = 8, 2048, 2048, 1024
CT = C // 128            # 8 contraction tiles
NF = 512                 # n-cols per attention chunk
NCH = N // NF            # 4 chunks
SCALE = (C // 8) ** -0.5
WS = 32.0                # host weight scale (keeps fp8 lo-split in normal range)
EXPSCALE = float(SCALE / (WS * WS))

_CACHE = {}


def _dr3(nc, ps, ah, al, bh, bl, first, last):
    """3-term fp8 DoubleRow accumulation block: (ah+al)^T(bh+bl) minus lo*lo.
    ah/al stationary slices [128, 2, <=128]; bh/bl moving [128, 2, <=256]."""
    terms = ((ah, bh), (ah, bl), (al, bh))
    for i, (a, b) in enumerate(terms):
        nc.tensor.matmul(
            ps, lhsT=a, rhs=b,
            start=(first and i == 0),
            stop=(last and i == 2),
            perf_mode=PM,
        )


def _build_program(mt_in):
    nc = bacc.Bacc(
        "TRN2",
        target_bir_lowering=False,
        debug=False,
        enable_asserts=False,
        num_devices=8,
    )

    x8h = nc.dram_tensor("x8h", [128, CT, N], F8, kind="ExternalInput")
    x8l = nc.dram_tensor("x8l", [128, CT, N], F8, kind="ExternalInput")
    s8h = nc.dram_tensor("s8h", [128, CT, M], F8, kind="ExternalInput")
    s8l = nc.dram_tensor("s8l", [128, CT, M], F8, kind="ExternalInput")
    wq8h = nc.dram_tensor("wq8h", [128, CT, C], F8, kind="ExternalInput")
    wq8l = nc.dram_tensor("wq8l", [128, CT, C], F8, kind="ExternalInput")
    wk8h = nc.dram_tensor("wk8h", [128, CT, C], F8, kind="ExternalInput")
    wk8l = nc.dram_tensor("wk8l", [128, CT, C], F8, kind="ExternalInput")
    wv8h = nc.dram_tensor("wv8h", [128, CT, C], F8, kind="ExternalInput")
    wv8l = nc.dram_tensor("wv8l", [128, CT, C], F8, kind="ExternalInput")
    pwb = nc.dram_tensor("pwb", [128, CT, C], BF16, kind="ExternalInput")
    maskf = nc.dram_tensor("maskf", [128, max(mt_in, 1)], F32,
                           kind="ExternalInput")
    biasb = nc.dram_tensor("biasb", [128, C], F32, kind="ExternalInput")
    out = nc.dram_tensor("out", [N, C], F32, kind="ExternalOutput")

    with tile.TileContext(nc, pool_alloc_mode="queue") as tc:
        _trace_kernel(tc, mt_in, x8h, x8l, s8h, s8l, wq8h, wq8l, wk8h, wk8l,
                      wv8h, wv8l, pwb, maskf, biasb, out)
    nc.compile()
    return nc


def _trace_kernel(tc, mt_in, x8h, x8l, s8h, s8l, wq8h, wq8l, wk8h, wk8l,
                  wv8h, wv8l, pwb, maskf, biasb, out):
    nc = tc.nc
    from contextlib import ExitStack

    MT = M // 128

    with ExitStack() as ctx:
        persist = ctx.enter_context(tc.tile_pool(name="persist", bufs=1))
        maskt = persist.tile([128, max(mt_in, 1)], F32, tag="maskt")
        nc.sync.dma_start(maskt[:], maskf[:])
        ones = persist.tile([128, 1], BF16, tag="ones")
        nc.vector.memset(ones[:], 1.0)
        bias = persist.tile([128, C], F32, tag="bias")
        nc.sync.dma_start(bias[:], biasb[:])

        # persistent activation-derived tensors
        kT = persist.tile([128, CT, M], BF16, tag="kT")
        v = persist.tile([128, max(mt_in, 1), C], BF16, tag="v")
        pwt = persist.tile([128, CT, C], BF16, tag="pwt")

        # ------------- build phase: kT and v (fp8 DoubleRow 3-term) -------
        with (
            tc.tile_pool(name="w8", bufs=1) as w8p,
            tc.tile_pool(name="sp8", bufs=1) as sp8,
            tc.tile_pool(name="bld", bufs=3, space="PSUM") as bld,
        ):
            wkh = w8p.tile([128, CT, C], F8, tag="wkh")
            wkl = w8p.tile([128, CT, C], F8, tag="wkl")
            wvh = w8p.tile([128, CT, C], F8, tag="wvh")
            wvl = w8p.tile([128, CT, C], F8, tag="wvl")
            sph = sp8.tile([128, CT, M], F8, tag="sph")
            spl = sp8.tile([128, CT, M], F8, tag="spl")
            # big coalesced DMAs (>=512B innermost runs), ordered so the first
            # k-groups' operands land first
            nc.sync.dma_start(wkh[:, 0:4, :], wk8h[:, 0:4, :])
            nc.sync.dma_start(wkl[:, 0:4, :], wk8l[:, 0:4, :])
            nc.sync.dma_start(sph[:, :, 0:512], s8h[:, :, 0:512])
            nc.sync.dma_start(spl[:, :, 0:512], s8l[:, :, 0:512])
            nc.sync.dma_start(wkh[:, 4:8, :], wk8h[:, 4:8, :])
            nc.sync.dma_start(wkl[:, 4:8, :], wk8l[:, 4:8, :])
            nc.sync.dma_start(wvh[:], wv8h[:])
            nc.sync.dma_start(wvl[:], wv8l[:])
            for mc in range(1, M // 512):
                sl = slice(mc * 512, (mc + 1) * 512)
                nc.sync.dma_start(sph[:, :, sl], s8h[:, :, sl])
                nc.sync.dma_start(spl[:, :, sl], s8l[:, :, sl])

            for mc in range(M // 256):
                sl = slice(mc * 256, (mc + 1) * 256)
                # kT[d, m] for this m-chunk: per d-tile
                for dt in range(CT):
                    ps = bld.tile([128, 512], F32, tag="bld")
                    for pr in range(CT // 2):
                        _dr3(nc, ps[:, 0:256],
                             wkh[:, 2 * pr:2 * pr + 2, dt * 128:(dt + 1) * 128],
                             wkl[:, 2 * pr:2 * pr + 2, dt * 128:(dt + 1) * 128],
                             sph[:, 2 * pr:2 * pr + 2, sl],
                             spl[:, 2 * pr:2 * pr + 2, sl],
                             pr == 0, pr == CT // 2 - 1)
                    nc.scalar.copy(kT[:, dt, sl], ps[:, 0:256])
                # v[m, d] for this chunk's masked-in m-tiles
                for j in range(2):
                    mt = mc * 2 + j
                    if mt >= mt_in:
                        continue
                    for dc in range(C // 256):
                        ps = bld.tile([128, 512], F32, tag="bld")
                        dsl = slice(dc * 256, (dc + 1) * 256)
                        for pr in range(CT // 2):
                            _dr3(nc, ps[:, 0:256],
                                 sph[:, 2 * pr:2 * pr + 2,
                                     mt * 128:(mt + 1) * 128],
                                 spl[:, 2 * pr:2 * pr + 2,
                                     mt * 128:(mt + 1) * 128],
                                 wvh[:, 2 * pr:2 * pr + 2, dsl],
                                 wvl[:, 2 * pr:2 * pr + 2, dsl],
                                 pr == 0, pr == CT // 2 - 1)
                        nc.vector.tensor_scalar_mul(
                            v[:, mt, dsl], ps[:, 0:256], maskt[:, mt:mt + 1])

        # ------------- attention + interleaved projection ------------------
        with (
            tc.tile_pool(name="wq8", bufs=1) as wq8p,
            tc.tile_pool(name="x8", bufs=2) as x8p,
            tc.tile_pool(name="qt", bufs=2) as qtp,
            tc.tile_pool(name="pt", bufs=18) as ptp,
            tc.tile_pool(name="ob", bufs=10) as obp,
            tc.tile_pool(name="fo", bufs=3) as fop,
            tc.tile_pool(name="st", bufs=4) as stp,
            tc.tile_pool(name="qps", bufs=2, space="PSUM") as qps,
            tc.tile_pool(name="sps", bufs=2, space="PSUM") as sps,
            tc.tile_pool(name="ops", bufs=2, space="PSUM") as ops,
            tc.tile_pool(name="djs", bufs=2, space="PSUM") as djs,
        ):
            wqh = wq8p.tile([128, CT, C], F8, tag="wqh")
            wql = wq8p.tile([128, CT, C], F8, tag="wql")
            nc.sync.dma_start(wqh[:], wq8h[:])
            nc.sync.dma_start(wql[:], wq8l[:])
            nc.sync.dma_start(pwt[:], pwb[:])

            x8 = [None, None]

            def load_x(c):
                xh = x8p.tile([128, CT, NF], F8, tag="xh")
                xl = x8p.tile([128, CT, NF], F8, tag="xl")
                nsl = slice(c * NF, (c + 1) * NF)
                nc.sync.dma_start(xh[:], x8h[:, :, nsl])
                nc.sync.dma_start(xl[:], x8l[:, :, nsl])
                return xh, xl

            x8[0] = load_x(0)
            x8[1] = load_x(1)

            out_v = out[:].rearrange("(t two) d -> two t d", two=2)
            o_half = [[None] * 8, [None] * 8]

            for c in range(NCH):
                xh, xl = x8[c % 2]
                # qT for this chunk (fp8 DoubleRow 3-term)
                qt = qtp.tile([128, CT, NF], BF16, tag="qt")
                for dt in range(CT):
                    for nh in range(NF // 256):
                        ps = qps.tile([128, 512], F32, tag="qps")
                        for pr in range(CT // 2):
                            _dr3(nc, ps[:, 0:256],
                                 wqh[:, 2 * pr:2 * pr + 2,
                                     dt * 128:(dt + 1) * 128],
                                 wql[:, 2 * pr:2 * pr + 2,
                                     dt * 128:(dt + 1) * 128],
                                 xh[:, 2 * pr:2 * pr + 2,
                                    nh * 256:(nh + 1) * 256],
                                 xl[:, 2 * pr:2 * pr + 2,
                                    nh * 256:(nh + 1) * 256],
                                 pr == 0, pr == CT // 2 - 1)
                        nc.scalar.copy(
                            qt[:, dt, nh * 256:(nh + 1) * 256], ps[:, 0:256])
                if c + 2 < NCH:
                    x8[c % 2] = load_x(c + 2)

                # sT per m-tile, exp -> pT (bf16)
                pts = []
                for mt in range(MT):
                    ps = sps.tile([128, NF], F32, tag="sps")
                    for dt in range(CT):
                        nc.tensor.matmul(
                            ps[:],
                            lhsT=kT[:, dt, mt * 128:(mt + 1) * 128],
                            rhs=qt[:, dt, :],
                            start=(dt == 0),
                            stop=(dt == CT - 1),
                        )
                    pt = ptp.tile([128, NF], BF16, tag="pt")
                    nc.scalar.activation(pt[:], ps[:], AF.Exp, scale=EXPSCALE)
                    pts.append(pt)

                # p@v + denominator + normalize, per n-tile of 128
                for nt in range(NF // 128):
                    ntile = c * (NF // 128) + nt
                    nsl = slice(nt * 128, (nt + 1) * 128)
                    o_ps = []
                    for dh in range(2):
                        ps = ops.tile([128, 512], F32, tag="ops")
                        for mt in range(mt_in):
                            nc.tensor.matmul(
                                ps[:],
                                lhsT=pts[mt][:, nsl],
                                rhs=v[:, mt, dh * 512:(dh + 1) * 512],
                                start=(mt == 0),
                                stop=(mt == mt_in - 1),
                            )
                        o_ps.append(ps)
                    dn = djs.tile([128, 512], F32, tag="djs")
                    for mt in range(MT):
                        nc.tensor.matmul(
                            dn[:, 0:1],
                            lhsT=pts[mt][:, nsl],
                            rhs=ones[:],
                            start=(mt == 0),
                            stop=(mt == MT - 1),
                        )
                    recip = stp.tile([128, 1], F32, tag="recip")
                    nc.vector.reciprocal(recip[:], dn[:, 0:1])
                    ob = obp.tile([128, C], BF16, tag="ob")
                    for dh in range(2):
                        nc.vector.tensor_scalar_mul(
                            ob[:, dh * 512:(dh + 1) * 512], o_ps[dh][:],
                            recip[:])
                    o_half[ntile // 8][ntile % 8] = ob

                # after each half: projection with the swapaxes fold
                if c % 2 == 1:
                    h = c // 2
                    otiles = o_half[h]
                    for tt in range(CT):
                        for dc in range(2):
                            ps = djs.tile([128, 512], F32, tag="djs")
                            for ct in range(CT):
                                nc.tensor.matmul(
                                    ps[:],
                                    lhsT=otiles[ct][:, tt * 128:(tt + 1) * 128],
                                    rhs=pwt[:, ct, dc * 512:(dc + 1) * 512],
                                    start=(ct == 0),
                                    stop=(ct == CT - 1),
                                )
                            f_sb = fop.tile([128, 512], F32, tag="fo")
                            nc.vector.tensor_add(
                                f_sb[:], ps[:], bias[:, dc * 512:(dc + 1) * 512])
                            nc.sync.dma_start(
                                out_v[h, tt * 128:(tt + 1) * 128,
                                      dc * 512:(dc + 1) * 512],
                                f_sb[:],
                            )


def _prep_layout(a):
    # a [rows(c), cols] -> [128, CT, cols] with c = ct*128 + p
    cols = a.shape[1]
    return np.ascontiguousarray(
        a.reshape(CT, 128, cols).transpose(1, 0, 2))


def _hl(a):
    hi = a.astype(F8NP)
    lo = (a - hi.astype(np.float32)).astype(F8NP)
    return np.ascontiguousarray(hi), np.ascontiguousarray(lo)


def prep_in_maps(x, support, attn_mask, qkv_w, proj_w, proj_b):
    x = np.asarray(x, dtype=np.float32)
    support = np.asarray(support, dtype=np.float32)
    attn_mask = np.asarray(attn_mask)
    qkv_w = np.asarray(qkv_w, dtype=np.float32)
    proj_w = np.asarray(proj_w, dtype=np.float32)
    proj_b = np.asarray(proj_b, dtype=np.float32)

    mask = (attn_mask != 0)
    perm = np.argsort(~mask, kind="stable")
    m1 = int(mask.sum())
    mt_in = max((m1 + 127) // 128, 1)
    mask_perm = mask[perm].astype(np.float32)

    wq = qkv_w[:C] * WS
    wk = qkv_w[C:2 * C] * WS
    wv = qkv_w[2 * C:] * WS
    wq8h, wq8l = _hl(_prep_layout(wq.T))
    wk8h, wk8l = _hl(_prep_layout(wk.T))
    wv8h, wv8l = _hl(_prep_layout(wv.T))
    pwb = np.ascontiguousarray(_prep_layout(proj_w.T).astype(BFNP))
    maskf = np.ascontiguousarray(
        (mask_perm[:mt_in * 128] / WS).reshape(mt_in, 128).T)
    biasb = np.ascontiguousarray(
        np.broadcast_to(proj_b, (128, C)).astype(np.float32))

    in_maps = []
    for b in range(B):
        x8h, x8l = _hl(_prep_layout(x[b].T))
        s8h, s8l = _hl(_prep_layout(support[b][perm].T))
        in_maps.append({
            "x8h": x8h, "x8l": x8l, "s8h": s8h, "s8l": s8l,
            "wq8h": wq8h, "wq8l": wq8l, "wk8h": wk8h, "wk8l": wk8l,
            "wv8h": wv8h, "wv8l": wv8l, "pwb": pwb,
            "maskf": maskf, "biasb": biasb,
        })
    return in_maps, mt_in


def kernel(x, support, attn_mask, qkv_w, proj_w, proj_b):
    in_maps, mt_in = prep_in_maps(x, support, attn_mask, qkv_w, proj_w,
                                  proj_b)
    if mt_in not in _CACHE:
        _CACHE[mt_in] = _build_program(mt_in)
    nc = _CACHE[mt_in]
    _CACHE["nc"] = nc

    res = run_bass_kernel_spmd(nc, in_maps, core_ids=list(range(B)))
    return np.stack([res.results[b]["out"] for b in range(B)], axis=0)


# revision 31
# speedup vs baseline: 1.2794x; 1.2794x over previous
"""Trainium2 Bass kernel for CrossAttention (B=8, N=M=2048, C=1024), fp32 io.

Sharding: data-parallel — one batch element per NeuronCore (8 cores).

Per-core pipeline (batch b), all matmuls bf16 (1 cyc/row) or fp8-e4m3
DoubleRow 3-term hi/lo (0.75x bf16 cost), fp32 PSUM accumulation:

  kT[d, m] = (support_perm @ wk^T)^T      fp8 DoubleRow, host-prepped operands
  v[m, d]  = mask_perm/32 * (support_perm @ wv^T)   (only mask=1 m-tiles)
  per n-chunk:
    qT[d, n] = (x @ wq^T)^T               fp8 DoubleRow
    sT[m, n] = kT^T qT   (psum, per m-tile; no transposes anywhere)
    pT = exp(SCALE' * sT) -> bf16         (no max-sub: logits <= ~16 in fp32)
    o[n, d] = sum over mask=1 m-tiles of pT^T @ v      (psum)
    den[n]  = ones-matmul column-sum of pT over ALL m-tiles (free-dim-1
              matmuls are ~free); o_sb = o * 1/den  -> bf16
  per n-half: proj with the swapaxes/reshape fold (contraction over o rows),
              bias add, DMA out.

support rows are permuted on host so mask=1 rows come first: the post-softmax
column mask makes masked-out columns contribute only to the softmax
denominator, so p@v and the v build skip them entirely (exact, not approx).
Weights are scaled x32 on host so fp8 hi/lo residuals stay in e4m3 normal
range; the 1/32 factors fold into the exp scale and the mask multiplier.
"""

import sys

sys.path.insert(0, "/opt/trn_rl_repo")

import numpy as np

import concourse.bass as bass
import concourse.tile as tile
from concourse import bacc, mybir
from concourse.bass_utils import run_bass_kernel_spmd

F32 = mybir.dt.float32
BF16 = mybir.dt.bfloat16
F8 = mybir.dt.float8e4
AF = mybir.ActivationFunctionType
PM = mybir.MatmulPerfMode.DoubleRow
F8NP = mybir.dt.np(F8)
BFNP = mybir.dt.np(BF16)

B, N, M, C = 8, 2048, 2048, 1024
CT = C // 128            # 8 contraction tiles
NF = 512                 # n-cols per attention chunk
NCH = N // NF            # 4 chunks
SCALE = (C // 8) ** -0.5
WS = 32.0                # host weight scale (keeps fp8 lo-split in normal range)
EXPSCALE = float(SCALE / (WS * WS))

_CACHE = {}


def _dr3(nc, ps, ah, al, bh, bl, first, last):
    """3-term fp8 DoubleRow accumulation block: (ah+al)^T(bh+bl) minus lo*lo.
    ah/al stationary slices [128, 2, <=128]; bh/bl moving [128, 2, <=256]."""
    terms = ((ah, bh), (ah, bl), (al, bh))
    for i, (a, b) in enumerate(terms):
        nc.tensor.matmul(
            ps, lhsT=a, rhs=b,
            start=(first and i == 0),
            stop=(last and i == 2),
            perf_mode=PM,
        )


def _build_program(mt_in):
    nc = bacc.Bacc(
        "TRN2",
        target_bir_lowering=False,
        debug=False,
        enable_asserts=False,
        num_devices=8,
    )

    x8h = nc.dram_tensor("x8h", [128, CT, N], F8, kind="ExternalInput")
    x8l = nc.dram_tensor("x8l", [128, CT, N], F8, kind="ExternalInput")
    s8h = nc.dram_tensor("s8h", [128, CT, M], F8, kind="ExternalInput")
    s8l = nc.dram_tensor("s8l", [128, CT, M], F8, kind="ExternalInput")
    wq8h = nc.dram_tensor("wq8h", [128, CT, C], F8, kind="ExternalInput")
    wq8l = nc.dram_tensor("wq8l", [128, CT, C], F8, kind="ExternalInput")
    wk8h = nc.dram_tensor("wk8h", [128, CT, C], F8, kind="ExternalInput")
    wk8l = nc.dram_tensor("wk8l", [128, CT, C], F8, kind="ExternalInput")
    wv8h = nc.dram_tensor("wv8h", [128, CT, C], F8, kind="ExternalInput")
    wv8l = nc.dram_tensor("wv8l", [128, CT, C], F8, kind="ExternalInput")
    pw8h = nc.dram_tensor("pw8h", [128, CT, C], F8, kind="ExternalInput")
    pw8l = nc.dram_tensor("pw8l", [128, CT, C], F8, kind="ExternalInput")
    maskf = nc.dram_tensor("maskf", [128, max(mt_in, 1)], F32,
                           kind="ExternalInput")
    biasb = nc.dram_tensor("biasb", [128, C], F32, kind="ExternalInput")
    out = nc.dram_tensor("out", [N, C], F32, kind="ExternalOutput")

    with tile.TileContext(nc, pool_alloc_mode="queue") as tc:
        _trace_kernel(tc, mt_in, x8h, x8l, s8h, s8l, wq8h, wq8l, wk8h, wk8l,
                      wv8h, wv8l, pw8h, pw8l, maskf, biasb, out)
    nc.compile()
    return nc


def _trace_kernel(tc, mt_in, x8h, x8l, s8h, s8l, wq8h, wq8l, wk8h, wk8l,
                  wv8h, wv8l, pw8h, pw8l, maskf, biasb, out):
    nc = tc.nc
    from contextlib import ExitStack

    MT = M // 128

    with ExitStack() as ctx:
        persist = ctx.enter_context(tc.tile_pool(name="persist", bufs=1))
        maskt = persist.tile([128, max(mt_in, 1)], F32, tag="maskt")
        ones = persist.tile([128, 1], BF16, tag="ones")
        nc.vector.memset(ones[:], 1.0)
        bias = persist.tile([128, C], F32, tag="bias")

        # persistent activation-derived tensors (fp8 hi/lo pairs)
        kth = persist.tile([128, CT, M], F8, tag="kth")
        ktl = persist.tile([128, CT, M], F8, tag="ktl")
        v = persist.tile([128, max(mt_in, 1), C], BF16, tag="v")
        pwh = persist.tile([128, CT, C], F8, tag="pwh")
        pwl = persist.tile([128, CT, C], F8, tag="pwl")

        # ------------- build phase: kT and v (fp8 DoubleRow 3-term) -------
        with (
            tc.tile_pool(name="w8", bufs=1) as w8p,
            tc.tile_pool(name="sp8", bufs=1) as sp8,
            tc.tile_pool(name="bld", bufs=6, space="PSUM") as bld,
        ):
            wkh = w8p.tile([128, CT, C], F8, tag="wkh")
            wkl = w8p.tile([128, CT, C], F8, tag="wkl")
            wvh = w8p.tile([128, CT, C], F8, tag="wvh")
            wvl = w8p.tile([128, CT, C], F8, tag="wvl")
            sph = sp8.tile([128, CT, M], F8, tag="sph")
            spl = sp8.tile([128, CT, M], F8, tag="spl")
            # big coalesced DMAs (>=512B innermost runs), ordered so the first
            # k-groups' operands land first; wv streams in while k(mc0..3) run
            nc.sync.dma_start(wkh[:, 0:4, :], wk8h[:, 0:4, :])
            nc.sync.dma_start(wkl[:, 0:4, :], wk8l[:, 0:4, :])
            nc.sync.dma_start(sph[:, :, 0:256], s8h[:, :, 0:256])
            nc.sync.dma_start(spl[:, :, 0:256], s8l[:, :, 0:256])
            nc.sync.dma_start(wkh[:, 4:8, :], wk8h[:, 4:8, :])
            nc.sync.dma_start(wkl[:, 4:8, :], wk8l[:, 4:8, :])
            nc.sync.dma_start(sph[:, :, 256:512], s8h[:, :, 256:512])
            nc.sync.dma_start(spl[:, :, 256:512], s8l[:, :, 256:512])
            nc.sync.dma_start(sph[:, :, 512:1024], s8h[:, :, 512:1024])
            nc.sync.dma_start(spl[:, :, 512:1024], s8l[:, :, 512:1024])
            nc.sync.dma_start(wvh[:], wv8h[:])
            nc.sync.dma_start(wvl[:], wv8l[:])
            nc.sync.dma_start(maskt[:], maskf[:])
            nc.sync.dma_start(bias[:], biasb[:])
            for mc in range(2, M // 512):
                sl = slice(mc * 512, (mc + 1) * 512)
                nc.sync.dma_start(sph[:, :, sl], s8h[:, :, sl])
                nc.sync.dma_start(spl[:, :, sl], s8l[:, :, sl])

            def k_chunk(mc):
                sl = slice(mc * 256, (mc + 1) * 256)
                for dt in range(CT):
                    ps = bld.tile([128, 512], F32, tag="bld")
                    for pr in range(CT // 2):
                        _dr3(nc, ps[:, 0:256],
                             wkh[:, 2 * pr:2 * pr + 2, dt * 128:(dt + 1) * 128],
                             wkl[:, 2 * pr:2 * pr + 2, dt * 128:(dt + 1) * 128],
                             sph[:, 2 * pr:2 * pr + 2, sl],
                             spl[:, 2 * pr:2 * pr + 2, sl],
                             pr == 0, pr == CT // 2 - 1)
                    nc.scalar.copy(kth[:, dt, sl], ps[:, 0:256])
                    nc.vector.tensor_sub(
                        ktl[:, dt, sl], ps[:, 0:256], kth[:, dt, sl])

            for mc in range(4):
                k_chunk(mc)
            for mt in range(mt_in):
                for dc in range(C // 256):
                    ps = bld.tile([128, 512], F32, tag="bld")
                    dsl = slice(dc * 256, (dc + 1) * 256)
                    for pr in range(CT // 2):
                        _dr3(nc, ps[:, 0:256],
                             sph[:, 2 * pr:2 * pr + 2,
                                 mt * 128:(mt + 1) * 128],
                             spl[:, 2 * pr:2 * pr + 2,
                                 mt * 128:(mt + 1) * 128],
                             wvh[:, 2 * pr:2 * pr + 2, dsl],
                             wvl[:, 2 * pr:2 * pr + 2, dsl],
                             pr == 0, pr == CT // 2 - 1)
                    nc.vector.tensor_scalar_mul(
                        v[:, mt, dsl], ps[:, 0:256], maskt[:, mt:mt + 1])
            for mc in range(4, M // 256):
                k_chunk(mc)

        # ------------- attention + interleaved projection ------------------
        with (
            tc.tile_pool(name="wq8", bufs=1) as wq8p,
            tc.tile_pool(name="x8", bufs=2) as x8p,
            tc.tile_pool(name="qt", bufs=2) as qtp,
            tc.tile_pool(name="pt", bufs=17) as ptp,
            tc.tile_pool(name="ob", bufs=4) as obp,
            tc.tile_pool(name="fo", bufs=7) as fop,
            tc.tile_pool(name="st", bufs=4) as stp,
            tc.tile_pool(name="sqs", bufs=4, space="PSUM") as sqs,
            tc.tile_pool(name="ops", bufs=2, space="PSUM") as ops,
            tc.tile_pool(name="djs", bufs=2, space="PSUM") as djs,
        ):
            wqh = wq8p.tile([128, CT, C], F8, tag="wqh")
            wql = wq8p.tile([128, CT, C], F8, tag="wql")
            nc.sync.dma_start(wqh[:], wq8h[:])
            nc.sync.dma_start(wql[:], wq8l[:])
            nc.sync.dma_start(pwh[:], pw8h[:])
            nc.sync.dma_start(pwl[:], pw8l[:])

            x8 = [None, None]

            def load_x(c):
                xh = x8p.tile([128, CT, NF], F8, tag="xh")
                xl = x8p.tile([128, CT, NF], F8, tag="xl")
                nsl = slice(c * NF, (c + 1) * NF)
                nc.sync.dma_start(xh[:], x8h[:, :, nsl])
                nc.sync.dma_start(xl[:], x8l[:, :, nsl])
                return xh, xl

            x8[0] = load_x(0)
            x8[1] = load_x(1)

            out_v = out[:].rearrange("(t two) d -> two t d", two=2)
            o_half = [None, None]
            pending = []

            def flush_normalize():
                # deferred DVE normalize of the previous n-tile: emitted after
                # the next chunk's q-phase so DVE's in-order queue can't block
                # the q psum recycling at the chunk boundary
                while pending:
                    o_ps, dn, ntile = pending.pop(0)
                    recip = stp.tile([128, 1], F32, tag="recip")
                    nc.vector.reciprocal(recip[:], dn[:, 0:1])
                    ohh, ohl = o_half[(ntile // 8) % 2]
                    j = ntile % 8
                    for dh in range(2):
                        dsl = slice(dh * 512, (dh + 1) * 512)
                        nc.vector.tensor_scalar_mul(
                            ohh[:, j, dsl], o_ps[dh][:], recip[:])
                        nc.vector.scalar_tensor_tensor(
                            ohl[:, j, dsl], o_ps[dh][:], recip[:],
                            ohh[:, j, dsl],
                            op0=mybir.AluOpType.mult,
                            op1=mybir.AluOpType.subtract)

            for c in range(NCH):
                xh, xl = x8[c % 2]
                # qT for this chunk (fp8 DoubleRow 3-term), split hi/lo
                qth = qtp.tile([128, CT, NF], F8, tag="qth")
                qtl = qtp.tile([128, CT, NF], F8, tag="qtl")
                for dt in range(CT):
                    for nh in range(NF // 256):
                        ps = sqs.tile([128, 512], F32, tag="sqs")
                        nsl = slice(nh * 256, (nh + 1) * 256)
                        for pr in range(CT // 2):
                            _dr3(nc, ps[:, 0:256],
                                 wqh[:, 2 * pr:2 * pr + 2,
                                     dt * 128:(dt + 1) * 128],
                                 wql[:, 2 * pr:2 * pr + 2,
                                     dt * 128:(dt + 1) * 128],
                                 xh[:, 2 * pr:2 * pr + 2, nsl],
                                 xl[:, 2 * pr:2 * pr + 2, nsl],
                                 pr == 0, pr == CT // 2 - 1)
                        nc.scalar.copy(qth[:, dt, nsl], ps[:, 0:256])
                        nc.vector.tensor_sub(
                            qtl[:, dt, nsl], ps[:, 0:256], qth[:, dt, nsl])
                if c + 2 < NCH:
                    x8[c % 2] = load_x(c + 2)

                # sT per (m-tile, n-half) fp8 DoubleRow 3-term, exp -> pT bf16
                pts = []
                for mt in range(MT):
                    pt = ptp.tile([128, NF], BF16, tag="pt")
                    for nh in range(NF // 256):
                        ps = sqs.tile([128, 512], F32, tag="sqs")
                        nsl = slice(nh * 256, (nh + 1) * 256)
                        for pr in range(CT // 2):
                            _dr3(nc, ps[:, 0:256],
                                 kth[:, 2 * pr:2 * pr + 2,
                                     mt * 128:(mt + 1) * 128],
                                 ktl[:, 2 * pr:2 * pr + 2,
                                     mt * 128:(mt + 1) * 128],
                                 qth[:, 2 * pr:2 * pr + 2, nsl],
                                 qtl[:, 2 * pr:2 * pr + 2, nsl],
                                 pr == 0, pr == CT // 2 - 1)
                        nc.scalar.activation(
                            pt[:, nsl], ps[:, 0:256], AF.Exp, scale=EXPSCALE)
                    pts.append(pt)

                # p@v + denominator + normalize, per n-tile of 128
                for nt in range(NF // 128):
                    ntile = c * (NF // 128) + nt
                    nsl = slice(nt * 128, (nt + 1) * 128)
                    o_ps = []
                    for dh in range(2):
                        ps = ops.tile([128, 512], F32, tag="ops")
                        for mt in range(mt_in):
                            nc.tensor.matmul(
                                ps[:],
                                lhsT=pts[mt][:, nsl],
                                rhs=v[:, mt, dh * 512:(dh + 1) * 512],
                                start=(mt == 0),
                                stop=(mt == mt_in - 1),
                            )
                        o_ps.append(ps)
                    dn = djs.tile([128, 512], F32, tag="djs")
                    for mt in range(MT):
                        nc.tensor.matmul(
                            dn[:, 0:1],
                            lhsT=pts[mt][:, nsl],
                            rhs=ones[:],
                            start=(mt == 0),
                            stop=(mt == MT - 1),
                        )
                    recip = stp.tile([128, 1], F32, tag="recip")
                    nc.vector.reciprocal(recip[:], dn[:, 0:1])
                    if ntile % 8 == 0:
                        ohh_new = obp.tile([128, 8, C], F8, tag="obh")
                        ohl_new = obp.tile([128, 8, C], F8, tag="obl")
                        o_half[(ntile // 8) % 2] = (ohh_new, ohl_new)
                    ohh, ohl = o_half[(ntile // 8) % 2]
                    j = ntile % 8
                    for dh in range(2):
                        dsl = slice(dh * 512, (dh + 1) * 512)
                        nc.vector.tensor_scalar_mul(
                            ohh[:, j, dsl], o_ps[dh][:], recip[:])
                        nc.vector.scalar_tensor_tensor(
                            ohl[:, j, dsl], o_ps[dh][:], recip[:],
                            ohh[:, j, dsl],
                            op0=mybir.AluOpType.mult,
                            op1=mybir.AluOpType.subtract)

                # after each half: projection with the swapaxes fold
                # (fp8 DoubleRow 3-term; pw scaled x32 on host, undone in the
                # bias add)
                if c % 2 == 1:
                    h = c // 2
                    ohh, ohl = o_half[h % 2]
                    for tt in range(CT):
                        for dc in range(C // 256):
                            ps = djs.tile([128, 512], F32, tag="djs")
                            dsl = slice(dc * 256, (dc + 1) * 256)
                            for pr in range(CT // 2):
                                _dr3(nc, ps[:, 0:256],
                                     ohh[:, 2 * pr:2 * pr + 2,
                                         tt * 128:(tt + 1) * 128],
                                     ohl[:, 2 * pr:2 * pr + 2,
                                         tt * 128:(tt + 1) * 128],
                                     pwh[:, 2 * pr:2 * pr + 2, dsl],
                                     pwl[:, 2 * pr:2 * pr + 2, dsl],
                                     pr == 0, pr == CT // 2 - 1)
                            f_sb = fop.tile([128, 256], F32, tag="fo")
                            nc.vector.scalar_tensor_tensor(
                                f_sb[:], ps[:, 0:256], 1.0 / WS,
                                bias[:, dsl],
                                op0=mybir.AluOpType.mult,
                                op1=mybir.AluOpType.add)
                            nc.sync.dma_start(
                                out_v[h, tt * 128:(tt + 1) * 128, dsl],
                                f_sb[:],
                            )


def _prep_layout(a):
    # a [rows(c), cols] -> [128, CT, cols] with c = ct*128 + p
    cols = a.shape[1]
    return np.ascontiguousarray(
        a.reshape(CT, 128, cols).transpose(1, 0, 2))


def _hl(a):
    hi = a.astype(F8NP)
    lo = (a - hi.astype(np.float32)).astype(F8NP)
    return np.ascontiguousarray(hi), np.ascontiguousarray(lo)


def prep_in_maps(x, support, attn_mask, qkv_w, proj_w, proj_b):
    x = np.asarray(x, dtype=np.float32)
    support = np.asarray(support, dtype=np.float32)
    attn_mask = np.asarray(attn_mask)
    qkv_w = np.asarray(qkv_w, dtype=np.float32)
    proj_w = np.asarray(proj_w, dtype=np.float32)
    proj_b = np.asarray(proj_b, dtype=np.float32)

    mask = (attn_mask != 0)
    perm = np.argsort(~mask, kind="stable")
    m1 = int(mask.sum())
    mt_in = max((m1 + 127) // 128, 1)
    mask_perm = mask[perm].astype(np.float32)

    wq = qkv_w[:C] * WS
    wk = qkv_w[C:2 * C] * WS
    wv = qkv_w[2 * C:] * WS
    wq8h, wq8l = _hl(_prep_layout(wq.T))
    wk8h, wk8l = _hl(_prep_layout(wk.T))
    wv8h, wv8l = _hl(_prep_layout(wv.T))
    pw8h, pw8l = _hl(_prep_layout(proj_w.T * WS))
    maskf = np.ascontiguousarray(
        (mask_perm[:mt_in * 128] / WS).reshape(mt_in, 128).T)
    biasb = np.ascontiguousarray(
        np.broadcast_to(proj_b, (128, C)).astype(np.float32))

    in_maps = []
    for b in range(B):
        x8h, x8l = _hl(_prep_layout(x[b].T))
        s8h, s8l = _hl(_prep_layout(support[b][perm].T))
        in_maps.append({
            "x8h": x8h, "x8l": x8l, "s8h": s8h, "s8l": s8l,
            "wq8h": wq8h, "wq8l": wq8l, "wk8h": wk8h, "wk8l": wk8l,
            "wv8h": wv8h, "wv8l": wv8l, "pw8h": pw8h, "pw8l": pw8l,
            "maskf": maskf, "biasb": biasb,
        })
    return in_maps, mt_in


def kernel(x, support, attn_mask, qkv_w, proj_w, proj_b):
    in_maps, mt_in = prep_in_maps(x, support, attn_mask, qkv_w, proj_w,
                                  proj_b)
    if mt_in not in _CACHE:
        _CACHE[mt_in] = _build_program(mt_in)
    nc = _CACHE[mt_in]
    _CACHE["nc"] = nc

    res = run_bass_kernel_spmd(nc, in_maps, core_ids=list(range(B)))
    return np.stack([res.results[b]["out"] for b in range(B)], axis=0)


# revision 39
# speedup vs baseline: 1.2858x; 1.0050x over previous
"""Trainium2 Bass kernel for CrossAttention (B=8, N=M=2048, C=1024), fp32 io.

Sharding: data-parallel — one batch element per NeuronCore (8 cores).

Per-core pipeline (batch b), all matmuls bf16 (1 cyc/row) or fp8-e4m3
DoubleRow 3-term hi/lo (0.75x bf16 cost), fp32 PSUM accumulation:

  kT[d, m] = (support_perm @ wk^T)^T      fp8 DoubleRow, host-prepped operands
  v[m, d]  = mask_perm/32 * (support_perm @ wv^T)   (only mask=1 m-tiles)
  per n-chunk:
    qT[d, n] = (x @ wq^T)^T               fp8 DoubleRow
    sT[m, n] = kT^T qT   (psum, per m-tile; no transposes anywhere)
    pT = exp(SCALE' * sT) -> bf16         (no max-sub: logits <= ~16 in fp32)
    o[n, d] = sum over mask=1 m-tiles of pT^T @ v      (psum)
    den[n]  = ones-matmul column-sum of pT over ALL m-tiles (free-dim-1
              matmuls are ~free); o_sb = o * 1/den  -> bf16
  per n-half: proj with the swapaxes/reshape fold (contraction over o rows),
              bias add, DMA out.

support rows are permuted on host so mask=1 rows come first: the post-softmax
column mask makes masked-out columns contribute only to the softmax
denominator, so p@v and the v build skip them entirely (exact, not approx).
Weights are scaled x32 on host so fp8 hi/lo residuals stay in e4m3 normal
range; the 1/32 factors fold into the exp scale and the mask multiplier.
"""

import sys

sys.path.insert(0, "/opt/trn_rl_repo")

import numpy as np

import concourse.bass as bass
import concourse.tile as tile
from concourse import bacc, mybir
from concourse.bass_utils import run_bass_kernel_spmd

F32 = mybir.dt.float32
BF16 = mybir.dt.bfloat16
F8 = mybir.dt.float8e4
AF = mybir.ActivationFunctionType
PM = mybir.MatmulPerfMode.DoubleRow
F8NP = mybir.dt.np(F8)
BFNP = mybir.dt.np(BF16)

B, N, M, C = 8, 2048, 2048, 1024
CT = C // 128            # 8 contraction tiles
NF = 512                 # n-cols per attention chunk
NCH = N // NF            # 4 chunks
SCALE = (C // 8) ** -0.5
WS = 32.0                # host weight scale (keeps fp8 lo-split in normal range)
EXPSCALE = float(SCALE / (WS * WS))

_CACHE = {}


def _dr3(nc, ps, ah, al, bh, bl, first, last):
    """3-term fp8 DoubleRow accumulation block: (ah+al)^T(bh+bl) minus lo*lo.
    ah/al stationary slices [128, 2, <=128]; bh/bl moving [128, 2, <=256]."""
    terms = ((ah, bh), (ah, bl), (al, bh))
    for i, (a, b) in enumerate(terms):
        nc.tensor.matmul(
            ps, lhsT=a, rhs=b,
            start=(first and i == 0),
            stop=(last and i == 2),
            perf_mode=PM,
        )


def _build_program(mt_in):
    nc = bacc.Bacc(
        "TRN2",
        target_bir_lowering=False,
        debug=False,
        enable_asserts=False,
        num_devices=8,
    )

    x8h = nc.dram_tensor("x8h", [128, CT, N], F8, kind="ExternalInput")
    x8l = nc.dram_tensor("x8l", [128, CT, N], F8, kind="ExternalInput")
    s8h = nc.dram_tensor("s8h", [128, CT, M], F8, kind="ExternalInput")
    s8l = nc.dram_tensor("s8l", [128, CT, M], F8, kind="ExternalInput")
    wq8h = nc.dram_tensor("wq8h", [128, CT, C], F8, kind="ExternalInput")
    wq8l = nc.dram_tensor("wq8l", [128, CT, C], F8, kind="ExternalInput")
    wk8h = nc.dram_tensor("wk8h", [128, CT, C], F8, kind="ExternalInput")
    wk8l = nc.dram_tensor("wk8l", [128, CT, C], F8, kind="ExternalInput")
    wv8h = nc.dram_tensor("wv8h", [128, CT, C], F8, kind="ExternalInput")
    wv8l = nc.dram_tensor("wv8l", [128, CT, C], F8, kind="ExternalInput")
    pw8h = nc.dram_tensor("pw8h", [128, CT, C], F8, kind="ExternalInput")
    pw8l = nc.dram_tensor("pw8l", [128, CT, C], F8, kind="ExternalInput")
    maskf = nc.dram_tensor("maskf", [128, max(mt_in, 1)], F32,
                           kind="ExternalInput")
    biasb = nc.dram_tensor("biasb", [128, C], F32, kind="ExternalInput")
    out = nc.dram_tensor("out", [N, C], F32, kind="ExternalOutput")

    with tile.TileContext(nc, pool_alloc_mode="queue") as tc:
        _trace_kernel(tc, mt_in, x8h, x8l, s8h, s8l, wq8h, wq8l, wk8h, wk8l,
                      wv8h, wv8l, pw8h, pw8l, maskf, biasb, out)
    nc.compile()
    return nc


def _trace_kernel(tc, mt_in, x8h, x8l, s8h, s8l, wq8h, wq8l, wk8h, wk8l,
                  wv8h, wv8l, pw8h, pw8l, maskf, biasb, out):
    nc = tc.nc
    from contextlib import ExitStack

    MT = M // 128

    with ExitStack() as ctx:
        persist = ctx.enter_context(tc.tile_pool(name="persist", bufs=1))
        maskt = persist.tile([128, max(mt_in, 1)], F32, tag="maskt")
        ones = persist.tile([128, 1], BF16, tag="ones")
        nc.vector.memset(ones[:], 1.0)
        bias = persist.tile([128, C], F32, tag="bias")

        # persistent activation-derived tensors (fp8 hi/lo pairs)
        kth = persist.tile([128, CT, M], F8, tag="kth")
        ktl = persist.tile([128, CT, M], F8, tag="ktl")
        v = persist.tile([128, max(mt_in, 1), C], BF16, tag="v")
        pwh = persist.tile([128, CT, C], F8, tag="pwh")
        pwl = persist.tile([128, CT, C], F8, tag="pwl")

        # ------------- build phase: kT and v (fp8 DoubleRow 3-term) -------
        with (
            tc.tile_pool(name="w8", bufs=1) as w8p,
            tc.tile_pool(name="sp8", bufs=1) as sp8,
            tc.tile_pool(name="bld", bufs=6, space="PSUM") as bld,
        ):
            wkh = w8p.tile([128, CT, C], F8, tag="wkh")
            wkl = w8p.tile([128, CT, C], F8, tag="wkl")
            wvh = w8p.tile([128, CT, C], F8, tag="wvh")
            wvl = w8p.tile([128, CT, C], F8, tag="wvl")
            sph = sp8.tile([128, CT, M], F8, tag="sph")
            spl = sp8.tile([128, CT, M], F8, tag="spl")
            # big coalesced DMAs (>=512B innermost runs), ordered so the first
            # k-groups' operands land first; wv streams in while k(mc0..3) run
            nc.sync.dma_start(wkh[:, 0:4, :], wk8h[:, 0:4, :])
            nc.sync.dma_start(wkl[:, 0:4, :], wk8l[:, 0:4, :])
            nc.sync.dma_start(sph[:, :, 0:256], s8h[:, :, 0:256])
            nc.sync.dma_start(spl[:, :, 0:256], s8l[:, :, 0:256])
            nc.sync.dma_start(wkh[:, 4:8, :], wk8h[:, 4:8, :])
            nc.sync.dma_start(wkl[:, 4:8, :], wk8l[:, 4:8, :])
            nc.sync.dma_start(sph[:, :, 256:512], s8h[:, :, 256:512])
            nc.sync.dma_start(spl[:, :, 256:512], s8l[:, :, 256:512])
            nc.sync.dma_start(sph[:, :, 512:1024], s8h[:, :, 512:1024])
            nc.sync.dma_start(spl[:, :, 512:1024], s8l[:, :, 512:1024])
            nc.sync.dma_start(wvh[:], wv8h[:])
            nc.sync.dma_start(wvl[:], wv8l[:])
            nc.sync.dma_start(maskt[:], maskf[:])
            nc.sync.dma_start(bias[:], biasb[:])
            for mc in range(2, M // 512):
                sl = slice(mc * 512, (mc + 1) * 512)
                nc.sync.dma_start(sph[:, :, sl], s8h[:, :, sl])
                nc.sync.dma_start(spl[:, :, sl], s8l[:, :, sl])

            def k_chunk(mc):
                sl = slice(mc * 256, (mc + 1) * 256)
                for dt in range(CT):
                    ps = bld.tile([128, 512], F32, tag="bld")
                    for pr in range(CT // 2):
                        _dr3(nc, ps[:, 0:256],
                             wkh[:, 2 * pr:2 * pr + 2, dt * 128:(dt + 1) * 128],
                             wkl[:, 2 * pr:2 * pr + 2, dt * 128:(dt + 1) * 128],
                             sph[:, 2 * pr:2 * pr + 2, sl],
                             spl[:, 2 * pr:2 * pr + 2, sl],
                             pr == 0, pr == CT // 2 - 1)
                    nc.scalar.copy(kth[:, dt, sl], ps[:, 0:256])
                    nc.vector.tensor_sub(
                        ktl[:, dt, sl], ps[:, 0:256], kth[:, dt, sl])

            for mc in range(M // 256):
                k_chunk(mc)
            for mt in range(mt_in):
                for dc in range(C // 256):
                    ps = bld.tile([128, 512], F32, tag="bld")
                    dsl = slice(dc * 256, (dc + 1) * 256)
                    for pr in range(CT // 2):
                        _dr3(nc, ps[:, 0:256],
                             sph[:, 2 * pr:2 * pr + 2,
                                 mt * 128:(mt + 1) * 128],
                             spl[:, 2 * pr:2 * pr + 2,
                                 mt * 128:(mt + 1) * 128],
                             wvh[:, 2 * pr:2 * pr + 2, dsl],
                             wvl[:, 2 * pr:2 * pr + 2, dsl],
                             pr == 0, pr == CT // 2 - 1)
                    nc.vector.tensor_scalar_mul(
                        v[:, mt, dsl], ps[:, 0:256], maskt[:, mt:mt + 1])

        # ------------- attention + interleaved projection ------------------
        with (
            tc.tile_pool(name="wq8", bufs=1) as wq8p,
            tc.tile_pool(name="x8", bufs=2) as x8p,
            tc.tile_pool(name="qt", bufs=2) as qtp,
            tc.tile_pool(name="pt", bufs=17) as ptp,
            tc.tile_pool(name="ob", bufs=4) as obp,
            tc.tile_pool(name="fo", bufs=7) as fop,
            tc.tile_pool(name="st", bufs=4) as stp,
            tc.tile_pool(name="sqs", bufs=4, space="PSUM") as sqs,
            tc.tile_pool(name="ops", bufs=2, space="PSUM") as ops,
            tc.tile_pool(name="djs", bufs=2, space="PSUM") as djs,
        ):
            wqh = wq8p.tile([128, CT, C], F8, tag="wqh")
            wql = wq8p.tile([128, CT, C], F8, tag="wql")
            nc.sync.dma_start(wqh[:], wq8h[:])
            nc.sync.dma_start(wql[:], wq8l[:])
            nc.sync.dma_start(pwh[:], pw8h[:])
            nc.sync.dma_start(pwl[:], pw8l[:])

            x8 = [None, None]

            def load_x(c):
                xh = x8p.tile([128, CT, NF], F8, tag="xh")
                xl = x8p.tile([128, CT, NF], F8, tag="xl")
                nsl = slice(c * NF, (c + 1) * NF)
                nc.sync.dma_start(xh[:], x8h[:, :, nsl])
                nc.sync.dma_start(xl[:], x8l[:, :, nsl])
                return xh, xl

            x8[0] = load_x(0)
            x8[1] = load_x(1)

            out_v = out[:].rearrange("(t two) d -> two t d", two=2)
            o_half = [None, None]
            pending = []

            def flush_normalize():
                # deferred DVE normalize of the previous n-tile: emitted after
                # the next chunk's q-phase so DVE's in-order queue can't block
                # the q psum recycling at the chunk boundary
                while pending:
                    o_ps, dn, ntile = pending.pop(0)
                    recip = stp.tile([128, 1], F32, tag="recip")
                    nc.vector.reciprocal(recip[:], dn[:, 0:1])
                    ohh, ohl = o_half[(ntile // 8) % 2]
                    j = ntile % 8
                    for dh in range(2):
                        dsl = slice(dh * 512, (dh + 1) * 512)
                        nc.scalar.activation(
                            ohh[:, j, dsl], o_ps[dh][:], AF.Copy,
                            scale=recip[:])
                        nc.vector.scalar_tensor_tensor(
                            ohl[:, j, dsl], o_ps[dh][:], recip[:],
                            ohh[:, j, dsl],
                            op0=mybir.AluOpType.mult,
                            op1=mybir.AluOpType.subtract)

            for c in range(NCH):
                xh, xl = x8[c % 2]
                # qT for this chunk (fp8 DoubleRow 3-term), split hi/lo
                qth = qtp.tile([128, CT, NF], F8, tag="qth")
                qtl = qtp.tile([128, CT, NF], F8, tag="qtl")
                for dt in range(CT):
                    for nh in range(NF // 256):
                        ps = sqs.tile([128, 512], F32, tag="sqs")
                        nsl = slice(nh * 256, (nh + 1) * 256)
                        for pr in range(CT // 2):
                            _dr3(nc, ps[:, 0:256],
                                 wqh[:, 2 * pr:2 * pr + 2,
                                     dt * 128:(dt + 1) * 128],
                                 wql[:, 2 * pr:2 * pr + 2,
                                     dt * 128:(dt + 1) * 128],
                                 xh[:, 2 * pr:2 * pr + 2, nsl],
                                 xl[:, 2 * pr:2 * pr + 2, nsl],
                                 pr == 0, pr == CT // 2 - 1)
                        nc.scalar.copy(qth[:, dt, nsl], ps[:, 0:256])
                        nc.vector.tensor_sub(
                            qtl[:, dt, nsl], ps[:, 0:256], qth[:, dt, nsl])
                if c + 2 < NCH:
                    x8[c % 2] = load_x(c + 2)
                flush_normalize()

                # sT per (m-tile, n-half) fp8 DoubleRow 3-term, exp -> pT bf16
                pts = []
                for mt in range(MT):
                    pt = ptp.tile([128, NF], BF16, tag="pt")
                    for nh in range(NF // 256):
                        ps = sqs.tile([128, 512], F32, tag="sqs")
                        nsl = slice(nh * 256, (nh + 1) * 256)
                        for pr in range(CT // 2):
                            _dr3(nc, ps[:, 0:256],
                                 kth[:, 2 * pr:2 * pr + 2,
                                     mt * 128:(mt + 1) * 128],
                                 ktl[:, 2 * pr:2 * pr + 2,
                                     mt * 128:(mt + 1) * 128],
                                 qth[:, 2 * pr:2 * pr + 2, nsl],
                                 qtl[:, 2 * pr:2 * pr + 2, nsl],
                                 pr == 0, pr == CT // 2 - 1)
                        nc.scalar.activation(
                            pt[:, nsl], ps[:, 0:256], AF.Exp, scale=EXPSCALE)
                    pts.append(pt)

                # p@v + denominator + normalize, per n-tile of 128
                for nt in range(NF // 128):
                    ntile = c * (NF // 128) + nt
                    nsl = slice(nt * 128, (nt + 1) * 128)
                    o_ps = []
                    for dh in range(2):
                        ps = ops.tile([128, 512], F32, tag="ops")
                        for mt in range(mt_in):
                            nc.tensor.matmul(
                                ps[:],
                                lhsT=pts[mt][:, nsl],
                                rhs=v[:, mt, dh * 512:(dh + 1) * 512],
                                start=(mt == 0),
                                stop=(mt == mt_in - 1),
                            )
                        o_ps.append(ps)
                    dn = djs.tile([128, 512], F32, tag="djs")
                    for mt in range(MT):
                        nc.tensor.matmul(
                            dn[:, 0:1],
                            lhsT=pts[mt][:, nsl],
                            rhs=ones[:],
                            start=(mt == 0),
                            stop=(mt == MT - 1),
                        )
                    if ntile % 8 == 0:
                        ohh_new = obp.tile([128, 8, C], F8, tag="obh")
                        ohl_new = obp.tile([128, 8, C], F8, tag="obl")
                        o_half[(ntile // 8) % 2] = (ohh_new, ohl_new)
                    if c % 2 == 0 and nt == NF // 128 - 1:
                        pending.append((o_ps, dn, ntile))
                        continue
                    recip = stp.tile([128, 1], F32, tag="recip")
                    nc.vector.reciprocal(recip[:], dn[:, 0:1])
                    ohh, ohl = o_half[(ntile // 8) % 2]
                    j = ntile % 8
                    for dh in range(2):
                        dsl = slice(dh * 512, (dh + 1) * 512)
                        nc.scalar.activation(
                            ohh[:, j, dsl], o_ps[dh][:], AF.Copy,
                            scale=recip[:])
                        nc.vector.scalar_tensor_tensor(
                            ohl[:, j, dsl], o_ps[dh][:], recip[:],
                            ohh[:, j, dsl],
                            op0=mybir.AluOpType.mult,
                            op1=mybir.AluOpType.subtract)

                # after each half: projection with the swapaxes fold
                # (fp8 DoubleRow 3-term; pw scaled x32 on host, undone in the
                # bias add)
                if c % 2 == 1:
                    h = c // 2
                    ohh, ohl = o_half[h % 2]
                    for tt in range(CT):
                        for dc in range(C // 256):
                            ps = djs.tile([128, 512], F32, tag="djs")
                            dsl = slice(dc * 256, (dc + 1) * 256)
                            for pr in range(CT // 2):
                                _dr3(nc, ps[:, 0:256],
                                     ohh[:, 2 * pr:2 * pr + 2,
                                         tt * 128:(tt + 1) * 128],
                                     ohl[:, 2 * pr:2 * pr + 2,
                                         tt * 128:(tt + 1) * 128],
                                     pwh[:, 2 * pr:2 * pr + 2, dsl],
                                     pwl[:, 2 * pr:2 * pr + 2, dsl],
                                     pr == 0, pr == CT // 2 - 1)
                            f_sb = fop.tile([128, 256], F32, tag="fo")
                            nc.vector.scalar_tensor_tensor(
                                f_sb[:], ps[:, 0:256], 1.0 / WS,
                                bias[:, dsl],
                                op0=mybir.AluOpType.mult,
                                op1=mybir.AluOpType.add)
                            nc.sync.dma_start(
                                out_v[h, tt * 128:(tt + 1) * 128, dsl],
                                f_sb[:],
                            )


def _prep_layout(a):
    # a [rows(c), cols] -> [128, CT, cols] with c = ct*128 + p
    cols = a.shape[1]
    return np.ascontiguousarray(
        a.reshape(CT, 128, cols).transpose(1, 0, 2))


def _hl(a):
    hi = a.astype(F8NP)
    lo = (a - hi.astype(np.float32)).astype(F8NP)
    return np.ascontiguousarray(hi), np.ascontiguousarray(lo)


def prep_in_maps(x, support, attn_mask, qkv_w, proj_w, proj_b):
    x = np.asarray(x, dtype=np.float32)
    support = np.asarray(support, dtype=np.float32)
    attn_mask = np.asarray(attn_mask)
    qkv_w = np.asarray(qkv_w, dtype=np.float32)
    proj_w = np.asarray(proj_w, dtype=np.float32)
    proj_b = np.asarray(proj_b, dtype=np.float32)

    mask = (attn_mask != 0)
    perm = np.argsort(~mask, kind="stable")
    m1 = int(mask.sum())
    mt_in = max((m1 + 127) // 128, 1)
    mask_perm = mask[perm].astype(np.float32)

    wq = qkv_w[:C] * WS
    wk = qkv_w[C:2 * C] * WS
    wv = qkv_w[2 * C:] * WS
    wq8h, wq8l = _hl(_prep_layout(wq.T))
    wk8h, wk8l = _hl(_prep_layout(wk.T))
    wv8h, wv8l = _hl(_prep_layout(wv.T))
    pw8h, pw8l = _hl(_prep_layout(proj_w.T * WS))
    maskf = np.ascontiguousarray(
        (mask_perm[:mt_in * 128] / WS).reshape(mt_in, 128).T)
    biasb = np.ascontiguousarray(
        np.broadcast_to(proj_b, (128, C)).astype(np.float32))

    in_maps = []
    for b in range(B):
        x8h, x8l = _hl(_prep_layout(x[b].T))
        s8h, s8l = _hl(_prep_layout(support[b][perm].T))
        in_maps.append({
            "x8h": x8h, "x8l": x8l, "s8h": s8h, "s8l": s8l,
            "wq8h": wq8h, "wq8l": wq8l, "wk8h": wk8h, "wk8l": wk8l,
            "wv8h": wv8h, "wv8l": wv8l, "pw8h": pw8h, "pw8l": pw8l,
            "maskf": maskf, "biasb": biasb,
        })
    return in_maps, mt_in


def kernel(x, support, attn_mask, qkv_w, proj_w, proj_b):
    in_maps, mt_in = prep_in_maps(x, support, attn_mask, qkv_w, proj_w,
                                  proj_b)
    if mt_in not in _CACHE:
        _CACHE[mt_in] = _build_program(mt_in)
    nc = _CACHE[mt_in]
    _CACHE["nc"] = nc

    res = run_bass_kernel_spmd(nc, in_maps, core_ids=list(range(B)))
    return np.stack([res.results[b]["out"] for b in range(B)], axis=0)


# revision 54
# speedup vs baseline: 1.2932x; 1.0058x over previous
"""Trainium2 Bass kernel for CrossAttention (B=8, N=M=2048, C=1024), fp32 io.

Sharding: data-parallel — one batch element per NeuronCore (8 cores).

Per-core pipeline (batch b), all matmuls bf16 (1 cyc/row) or fp8-e4m3
DoubleRow 3-term hi/lo (0.75x bf16 cost), fp32 PSUM accumulation:

  kT[d, m] = (support_perm @ wk^T)^T      fp8 DoubleRow, host-prepped operands
  v[m, d]  = mask_perm/32 * (support_perm @ wv^T)   (only mask=1 m-tiles)
  per n-chunk:
    qT[d, n] = (x @ wq^T)^T               fp8 DoubleRow
    sT[m, n] = kT^T qT   (psum, per m-tile; no transposes anywhere)
    pT = exp(SCALE' * sT) -> bf16         (no max-sub: logits <= ~16 in fp32)
    o[n, d] = sum over mask=1 m-tiles of pT^T @ v      (psum)
    den[n]  = ones-matmul column-sum of pT over ALL m-tiles (free-dim-1
              matmuls are ~free); o_sb = o * 1/den  -> bf16
  per n-half: proj with the swapaxes/reshape fold (contraction over o rows),
              bias add, DMA out.

support rows are permuted on host so mask=1 rows come first: the post-softmax
column mask makes masked-out columns contribute only to the softmax
denominator, so p@v and the v build skip them entirely (exact, not approx).
Weights are scaled x32 on host so fp8 hi/lo residuals stay in e4m3 normal
range; the 1/32 factors fold into the exp scale and the mask multiplier.
"""

import sys

sys.path.insert(0, "/opt/trn_rl_repo")

import numpy as np

import concourse.bass as bass
import concourse.tile as tile
from concourse import bacc, mybir
from concourse.bass_utils import run_bass_kernel_spmd

F32 = mybir.dt.float32
BF16 = mybir.dt.bfloat16
F8 = mybir.dt.float8e4
AF = mybir.ActivationFunctionType
PM = mybir.MatmulPerfMode.DoubleRow
F8NP = mybir.dt.np(F8)
BFNP = mybir.dt.np(BF16)

B, N, M, C = 8, 2048, 2048, 1024
CT = C // 128            # 8 contraction tiles
NF = 512                 # n-cols per attention chunk
NCH = N // NF            # 4 chunks
SCALE = (C // 8) ** -0.5
WS = 32.0                # host weight scale (keeps fp8 lo-split in normal range)
EXPSCALE = float(SCALE / (WS * WS))

_CACHE = {}


def _dr3(nc, ps, ah, al, bh, bl, first, last):
    """3-term fp8 DoubleRow accumulation block: (ah+al)^T(bh+bl) minus lo*lo.
    ah/al stationary slices [128, 2, <=128]; bh/bl moving [128, 2, <=256]."""
    terms = ((ah, bh), (ah, bl), (al, bh))
    for i, (a, b) in enumerate(terms):
        nc.tensor.matmul(
            ps, lhsT=a, rhs=b,
            start=(first and i == 0),
            stop=(last and i == 2),
            perf_mode=PM,
        )


def _build_program(mt_in):
    nc = bacc.Bacc(
        "TRN2",
        target_bir_lowering=False,
        debug=False,
        enable_asserts=False,
        num_devices=8,
    )

    x8h = nc.dram_tensor("x8h", [128, CT, N], F8, kind="ExternalInput")
    x8l = nc.dram_tensor("x8l", [128, CT, N], F8, kind="ExternalInput")
    s8h = nc.dram_tensor("s8h", [128, CT, M], F8, kind="ExternalInput")
    s8l = nc.dram_tensor("s8l", [128, CT, M], F8, kind="ExternalInput")
    wq8h = nc.dram_tensor("wq8h", [128, CT, C], F8, kind="ExternalInput")
    wq8l = nc.dram_tensor("wq8l", [128, CT, C], F8, kind="ExternalInput")
    # wk is d-tile-major so per-dt DMA slices are contiguous: the first
    # k-group only needs 1/8th of the weight bytes before starting
    wk8h = nc.dram_tensor("wk8h", [128, CT, CT, 128], F8, kind="ExternalInput")
    wk8l = nc.dram_tensor("wk8l", [128, CT, CT, 128], F8, kind="ExternalInput")
    wv8h = nc.dram_tensor("wv8h", [128, CT, C], F8, kind="ExternalInput")
    wv8l = nc.dram_tensor("wv8l", [128, CT, C], F8, kind="ExternalInput")
    pw8h = nc.dram_tensor("pw8h", [128, CT, C], F8, kind="ExternalInput")
    pw8l = nc.dram_tensor("pw8l", [128, CT, C], F8, kind="ExternalInput")
    maskf = nc.dram_tensor("maskf", [128, max(mt_in, 1)], F32,
                           kind="ExternalInput")
    biasb = nc.dram_tensor("biasb", [128, C], F32, kind="ExternalInput")
    out = nc.dram_tensor("out", [N, C], F32, kind="ExternalOutput")

    with tile.TileContext(nc, pool_alloc_mode="queue") as tc:
        _trace_kernel(tc, mt_in, x8h, x8l, s8h, s8l, wq8h, wq8l, wk8h, wk8l,
                      wv8h, wv8l, pw8h, pw8l, maskf, biasb, out)
    nc.compile()
    return nc


def _trace_kernel(tc, mt_in, x8h, x8l, s8h, s8l, wq8h, wq8l, wk8h, wk8l,
                  wv8h, wv8l, pw8h, pw8l, maskf, biasb, out):
    nc = tc.nc
    from contextlib import ExitStack

    MT = M // 128

    with ExitStack() as ctx:
        persist = ctx.enter_context(tc.tile_pool(name="persist", bufs=1))
        maskt = persist.tile([128, max(mt_in, 1)], F32, tag="maskt")
        ones = persist.tile([128, 1], BF16, tag="ones")
        nc.vector.memset(ones[:], 1.0)
        bias = persist.tile([128, C], F32, tag="bias")

        # persistent activation-derived tensors (fp8 hi/lo pairs)
        kth = persist.tile([128, CT, M], F8, tag="kth")
        ktl = persist.tile([128, CT, M], F8, tag="ktl")
        v = persist.tile([128, max(mt_in, 1), C], BF16, tag="v")
        pwh = persist.tile([128, CT, C], F8, tag="pwh")
        pwl = persist.tile([128, CT, C], F8, tag="pwl")

        # ------------- build phase: kT and v (fp8 DoubleRow 3-term) -------
        with (
            tc.tile_pool(name="w8", bufs=1) as w8p,
            tc.tile_pool(name="sp8", bufs=1) as sp8,
            tc.tile_pool(name="bld", bufs=6, space="PSUM") as bld,
        ):
            wkh = w8p.tile([128, CT, CT, 128], F8, tag="wkh")
            wkl = w8p.tile([128, CT, CT, 128], F8, tag="wkl")
            wvh = w8p.tile([128, CT, C], F8, tag="wvh")
            wvl = w8p.tile([128, CT, C], F8, tag="wvl")
            sph = sp8.tile([128, CT, M], F8, tag="sph")
            spl = sp8.tile([128, CT, M], F8, tag="spl")
            # big coalesced DMAs (>=512B innermost runs), ordered so the first
            # k-group's operands (wk d-tile 0 + first support chunk) land first
            nc.sync.dma_start(wkh[:, 0, :, :], wk8h[:, 0, :, :])
            nc.sync.dma_start(wkl[:, 0, :, :], wk8l[:, 0, :, :])
            nc.sync.dma_start(sph[:, :, 0:512], s8h[:, :, 0:512])
            nc.sync.dma_start(spl[:, :, 0:512], s8l[:, :, 0:512])
            for dt in range(1, CT):
                nc.sync.dma_start(wkh[:, dt, :, :], wk8h[:, dt, :, :])
                nc.sync.dma_start(wkl[:, dt, :, :], wk8l[:, dt, :, :])
            nc.sync.dma_start(sph[:, :, 512:1024], s8h[:, :, 512:1024])
            nc.sync.dma_start(spl[:, :, 512:1024], s8l[:, :, 512:1024])
            nc.sync.dma_start(wvh[:], wv8h[:])
            nc.sync.dma_start(wvl[:], wv8l[:])
            nc.sync.dma_start(maskt[:], maskf[:])
            nc.sync.dma_start(bias[:], biasb[:])
            for mc in range(2, M // 512):
                sl = slice(mc * 512, (mc + 1) * 512)
                nc.sync.dma_start(sph[:, :, sl], s8h[:, :, sl])
                nc.sync.dma_start(spl[:, :, sl], s8l[:, :, sl])

            def k_group(mc, dt):
                sl = slice(mc * 256, (mc + 1) * 256)
                ps = bld.tile([128, 512], F32, tag="bld")
                for pr in range(CT // 2):
                    _dr3(nc, ps[:, 0:256],
                         wkh[:, dt, 2 * pr:2 * pr + 2, :],
                         wkl[:, dt, 2 * pr:2 * pr + 2, :],
                         sph[:, 2 * pr:2 * pr + 2, sl],
                         spl[:, 2 * pr:2 * pr + 2, sl],
                         pr == 0, pr == CT // 2 - 1)
                nc.scalar.copy(kth[:, dt, sl], ps[:, 0:256])
                nc.vector.tensor_sub(
                    ktl[:, dt, sl], ps[:, 0:256], kth[:, dt, sl])

            # dt-major over the first two m-chunks: each wk d-tile is consumed
            # twice per arrival, so the PE builds slack against the DMA stream
            for dt in range(CT):
                k_group(0, dt)
                k_group(1, dt)
            for mc in range(2, M // 256):
                for dt in range(CT):
                    k_group(mc, dt)
            for mt in range(mt_in):
                for dc in range(C // 256):
                    ps = bld.tile([128, 512], F32, tag="bld")
                    dsl = slice(dc * 256, (dc + 1) * 256)
                    for pr in range(CT // 2):
                        _dr3(nc, ps[:, 0:256],
                             sph[:, 2 * pr:2 * pr + 2,
                                 mt * 128:(mt + 1) * 128],
                             spl[:, 2 * pr:2 * pr + 2,
                                 mt * 128:(mt + 1) * 128],
                             wvh[:, 2 * pr:2 * pr + 2, dsl],
                             wvl[:, 2 * pr:2 * pr + 2, dsl],
                             pr == 0, pr == CT // 2 - 1)
                    nc.vector.tensor_scalar_mul(
                        v[:, mt, dsl], ps[:, 0:256], maskt[:, mt:mt + 1])

        # ------------- attention + interleaved projection ------------------
        with (
            tc.tile_pool(name="wq8", bufs=1) as wq8p,
            tc.tile_pool(name="x8", bufs=2) as x8p,
            tc.tile_pool(name="qt", bufs=2) as qtp,
            tc.tile_pool(name="pt", bufs=17) as ptp,
            tc.tile_pool(name="ob", bufs=4) as obp,
            tc.tile_pool(name="fo", bufs=7) as fop,
            tc.tile_pool(name="st", bufs=4) as stp,
            tc.tile_pool(name="sqs", bufs=4, space="PSUM") as sqs,
            tc.tile_pool(name="ops", bufs=2, space="PSUM") as ops,
            tc.tile_pool(name="djs", bufs=2, space="PSUM") as djs,
        ):
            wqh = wq8p.tile([128, CT, C], F8, tag="wqh")
            wql = wq8p.tile([128, CT, C], F8, tag="wql")
            nc.sync.dma_start(wqh[:], wq8h[:])
            nc.sync.dma_start(wql[:], wq8l[:])
            nc.sync.dma_start(pwh[:], pw8h[:])
            nc.sync.dma_start(pwl[:], pw8l[:])

            x8 = [None, None]

            def load_x(c):
                xh = x8p.tile([128, CT, NF], F8, tag="xh")
                xl = x8p.tile([128, CT, NF], F8, tag="xl")
                nsl = slice(c * NF, (c + 1) * NF)
                nc.sync.dma_start(xh[:], x8h[:, :, nsl])
                nc.sync.dma_start(xl[:], x8l[:, :, nsl])
                return xh, xl

            x8[0] = load_x(0)
            x8[1] = load_x(1)

            out_v = out[:].rearrange("(t two) d -> two t d", two=2)
            o_half = [None, None]
            pending = []

            def flush_normalize():
                # deferred DVE normalize of the previous n-tile: emitted after
                # the next chunk's q-phase so DVE's in-order queue can't block
                # the q psum recycling at the chunk boundary
                while pending:
                    o_ps, dn, ntile = pending.pop(0)
                    recip = stp.tile([128, 1], F32, tag="recip")
                    nc.vector.reciprocal(recip[:], dn[:, 0:1])
                    ohh, ohl = o_half[(ntile // 8) % 2]
                    j = ntile % 8
                    for dh in range(2):
                        dsl = slice(dh * 512, (dh + 1) * 512)
                        nc.scalar.activation(
                            ohh[:, j, dsl], o_ps[dh][:], AF.Copy,
                            scale=recip[:])
                        nc.vector.scalar_tensor_tensor(
                            ohl[:, j, dsl], o_ps[dh][:], recip[:],
                            ohh[:, j, dsl],
                            op0=mybir.AluOpType.mult,
                            op1=mybir.AluOpType.subtract)

            for c in range(NCH):
                xh, xl = x8[c % 2]
                # qT for this chunk (fp8 DoubleRow 3-term), split hi/lo
                qth = qtp.tile([128, CT, NF], F8, tag="qth")
                qtl = qtp.tile([128, CT, NF], F8, tag="qtl")
                for dt in range(CT):
                    for nh in range(NF // 256):
                        ps = sqs.tile([128, 512], F32, tag="sqs")
                        nsl = slice(nh * 256, (nh + 1) * 256)
                        for pr in range(CT // 2):
                            _dr3(nc, ps[:, 0:256],
                                 wqh[:, 2 * pr:2 * pr + 2,
                                     dt * 128:(dt + 1) * 128],
                                 wql[:, 2 * pr:2 * pr + 2,
                                     dt * 128:(dt + 1) * 128],
                                 xh[:, 2 * pr:2 * pr + 2, nsl],
                                 xl[:, 2 * pr:2 * pr + 2, nsl],
                                 pr == 0, pr == CT // 2 - 1)
                        nc.scalar.copy(qth[:, dt, nsl], ps[:, 0:256])
                        nc.vector.tensor_sub(
                            qtl[:, dt, nsl], ps[:, 0:256], qth[:, dt, nsl])
                if c + 2 < NCH:
                    x8[c % 2] = load_x(c + 2)
                flush_normalize()

                # sT per (m-tile, n-half) fp8 DoubleRow 3-term, exp -> pT bf16
                pts = []
                for mt in range(MT):
                    pt = ptp.tile([128, NF], BF16, tag="pt")
                    for nh in range(NF // 256):
                        ps = sqs.tile([128, 512], F32, tag="sqs")
                        nsl = slice(nh * 256, (nh + 1) * 256)
                        for pr in range(CT // 2):
                            _dr3(nc, ps[:, 0:256],
                                 kth[:, 2 * pr:2 * pr + 2,
                                     mt * 128:(mt + 1) * 128],
                                 ktl[:, 2 * pr:2 * pr + 2,
                                     mt * 128:(mt + 1) * 128],
                                 qth[:, 2 * pr:2 * pr + 2, nsl],
                                 qtl[:, 2 * pr:2 * pr + 2, nsl],
                                 pr == 0, pr == CT // 2 - 1)
                        nc.scalar.activation(
                            pt[:, nsl], ps[:, 0:256], AF.Exp, scale=EXPSCALE)
                    pts.append(pt)

                # p@v + denominator + normalize, per n-tile of 128
                for nt in range(NF // 128):
                    ntile = c * (NF // 128) + nt
                    nsl = slice(nt * 128, (nt + 1) * 128)
                    o_ps = []
                    for dh in range(2):
                        ps = ops.tile([128, 512], F32, tag="ops")
                        for mt in range(mt_in):
                            nc.tensor.matmul(
                                ps[:],
                                lhsT=pts[mt][:, nsl],
                                rhs=v[:, mt, dh * 512:(dh + 1) * 512],
                                start=(mt == 0),
                                stop=(mt == mt_in - 1),
                            )
                        o_ps.append(ps)
                    dn = djs.tile([128, 512], F32, tag="djs")
                    for mt in range(MT):
                        nc.tensor.matmul(
                            dn[:, 0:1],
                            lhsT=pts[mt][:, nsl],
                            rhs=ones[:],
                            start=(mt == 0),
                            stop=(mt == MT - 1),
                        )
                    if ntile % 8 == 0:
                        ohh_new = obp.tile([128, 8, C], F8, tag="obh")
                        ohl_new = obp.tile([128, 8, C], F8, tag="obl")
                        o_half[(ntile // 8) % 2] = (ohh_new, ohl_new)
                    if c % 2 == 0 and nt == NF // 128 - 1:
                        pending.append((o_ps, dn, ntile))
                        continue
                    recip = stp.tile([128, 1], F32, tag="recip")
                    nc.vector.reciprocal(recip[:], dn[:, 0:1])
                    ohh, ohl = o_half[(ntile // 8) % 2]
                    j = ntile % 8
                    for dh in range(2):
                        dsl = slice(dh * 512, (dh + 1) * 512)
                        nc.scalar.activation(
                            ohh[:, j, dsl], o_ps[dh][:], AF.Copy,
                            scale=recip[:])
                        nc.vector.scalar_tensor_tensor(
                            ohl[:, j, dsl], o_ps[dh][:], recip[:],
                            ohh[:, j, dsl],
                            op0=mybir.AluOpType.mult,
                            op1=mybir.AluOpType.subtract)

                # after each half: projection with the swapaxes fold
                # (fp8 DoubleRow 3-term; pw scaled x32 on host, undone in the
                # bias add)
                if c % 2 == 1:
                    h = c // 2
                    ohh, ohl = o_half[h % 2]
                    for tt in range(CT):
                        for dc in range(C // 256):
                            ps = djs.tile([128, 512], F32, tag="djs")
                            dsl = slice(dc * 256, (dc + 1) * 256)
                            for pr in range(CT // 2):
                                _dr3(nc, ps[:, 0:256],
                                     ohh[:, 2 * pr:2 * pr + 2,
                                         tt * 128:(tt + 1) * 128],
                                     ohl[:, 2 * pr:2 * pr + 2,
                                         tt * 128:(tt + 1) * 128],
                                     pwh[:, 2 * pr:2 * pr + 2, dsl],
                                     pwl[:, 2 * pr:2 * pr + 2, dsl],
                                     pr == 0, pr == CT // 2 - 1)
                            f_sb = fop.tile([128, 256], F32, tag="fo")
                            nc.vector.scalar_tensor_tensor(
                                f_sb[:], ps[:, 0:256], 1.0 / WS,
                                bias[:, dsl],
                                op0=mybir.AluOpType.mult,
                                op1=mybir.AluOpType.add)
                            nc.sync.dma_start(
                                out_v[h, tt * 128:(tt + 1) * 128, dsl],
                                f_sb[:],
                            )


def _prep_layout(a):
    # a [rows(c), cols] -> [128, CT, cols] with c = ct*128 + p
    cols = a.shape[1]
    return np.ascontiguousarray(
        a.reshape(CT, 128, cols).transpose(1, 0, 2))


def _prep_layout_dt(a):
    # a [rows(c), cols(d)] -> [128, CT(dt), CT(ct), 128] with c = ct*128 + p,
    # d = dt*128 + dd (d-tile-major for per-dt contiguous DMA slices)
    return np.ascontiguousarray(
        a.reshape(CT, 128, CT, 128).transpose(1, 2, 0, 3))


def _hl(a):
    hi = a.astype(F8NP)
    lo = (a - hi.astype(np.float32)).astype(F8NP)
    return np.ascontiguousarray(hi), np.ascontiguousarray(lo)


def prep_in_maps(x, support, attn_mask, qkv_w, proj_w, proj_b):
    x = np.asarray(x, dtype=np.float32)
    support = np.asarray(support, dtype=np.float32)
    attn_mask = np.asarray(attn_mask)
    qkv_w = np.asarray(qkv_w, dtype=np.float32)
    proj_w = np.asarray(proj_w, dtype=np.float32)
    proj_b = np.asarray(proj_b, dtype=np.float32)

    mask = (attn_mask != 0)
    perm = np.argsort(~mask, kind="stable")
    m1 = int(mask.sum())
    mt_in = max((m1 + 127) // 128, 1)
    mask_perm = mask[perm].astype(np.float32)

    wq = qkv_w[:C] * WS
    wk = qkv_w[C:2 * C] * WS
    wv = qkv_w[2 * C:] * WS
    wq8h, wq8l = _hl(_prep_layout(wq.T))
    wk8h, wk8l = _hl(_prep_layout_dt(wk.T))
    wv8h, wv8l = _hl(_prep_layout(wv.T))
    pw8h, pw8l = _hl(_prep_layout(proj_w.T * WS))
    maskf = np.ascontiguousarray(
        (mask_perm[:mt_in * 128] / WS).reshape(mt_in, 128).T)
    biasb = np.ascontiguousarray(
        np.broadcast_to(proj_b, (128, C)).astype(np.float32))

    in_maps = []
    for b in range(B):
        x8h, x8l = _hl(_prep_layout(x[b].T))
        s8h, s8l = _hl(_prep_layout(support[b][perm].T))
        in_maps.append({
            "x8h": x8h, "x8l": x8l, "s8h": s8h, "s8l": s8l,
            "wq8h": wq8h, "wq8l": wq8l, "wk8h": wk8h, "wk8l": wk8l,
            "wv8h": wv8h, "wv8l": wv8l, "pw8h": pw8h, "pw8l": pw8l,
            "maskf": maskf, "biasb": biasb,
        })
    return in_maps, mt_in


def kernel(x, support, attn_mask, qkv_w, proj_w, proj_b):
    in_maps, mt_in = prep_in_maps(x, support, attn_mask, qkv_w, proj_w,
                                  proj_b)
    if mt_in not in _CACHE:
        _CACHE[mt_in] = _build_program(mt_in)
    nc = _CACHE[mt_in]
    _CACHE["nc"] = nc

    res = run_bass_kernel_spmd(nc, in_maps, core_ids=list(range(B)))
    return np.stack([res.results[b]["out"] for b in range(B)], axis=0)


# revision 57
# speedup vs baseline: 1.3011x; 1.0060x over previous
"""Trainium2 Bass kernel for CrossAttention (B=8, N=M=2048, C=1024), fp32 io.

Sharding: data-parallel — one batch element per NeuronCore (8 cores).

Per-core pipeline (batch b), all matmuls bf16 (1 cyc/row) or fp8-e4m3
DoubleRow 3-term hi/lo (0.75x bf16 cost), fp32 PSUM accumulation:

  kT[d, m] = (support_perm @ wk^T)^T      fp8 DoubleRow, host-prepped operands
  v[m, d]  = mask_perm/32 * (support_perm @ wv^T)   (only mask=1 m-tiles)
  per n-chunk:
    qT[d, n] = (x @ wq^T)^T               fp8 DoubleRow
    sT[m, n] = kT^T qT   (psum, per m-tile; no transposes anywhere)
    pT = exp(SCALE' * sT) -> bf16         (no max-sub: logits <= ~16 in fp32)
    o[n, d] = sum over mask=1 m-tiles of pT^T @ v      (psum)
    den[n]  = ones-matmul column-sum of pT over ALL m-tiles (free-dim-1
              matmuls are ~free); o_sb = o * 1/den  -> bf16
  per n-half: proj with the swapaxes/reshape fold (contraction over o rows),
              bias add, DMA out.

support rows are permuted on host so mask=1 rows come first: the post-softmax
column mask makes masked-out columns contribute only to the softmax
denominator, so p@v and the v build skip them entirely (exact, not approx).
Weights are scaled x32 on host so fp8 hi/lo residuals stay in e4m3 normal
range; the 1/32 factors fold into the exp scale and the mask multiplier.
"""

import sys

sys.path.insert(0, "/opt/trn_rl_repo")

import numpy as np

import concourse.bass as bass
import concourse.tile as tile
from concourse import bacc, mybir
from concourse.bass_utils import run_bass_kernel_spmd

F32 = mybir.dt.float32
BF16 = mybir.dt.bfloat16
F8 = mybir.dt.float8e4
AF = mybir.ActivationFunctionType
PM = mybir.MatmulPerfMode.DoubleRow
F8NP = mybir.dt.np(F8)
BFNP = mybir.dt.np(BF16)

B, N, M, C = 8, 2048, 2048, 1024
CT = C // 128            # 8 contraction tiles
NF = 512                 # n-cols per attention chunk
NCH = N // NF            # 4 chunks
SCALE = (C // 8) ** -0.5
WS = 32.0                # host weight scale (keeps fp8 lo-split in normal range)
EXPSCALE = float(SCALE / (WS * WS))

_CACHE = {}


def _dr3(nc, ps, ah, al, bh, bl, first, last):
    """3-term fp8 DoubleRow accumulation block: (ah+al)^T(bh+bl) minus lo*lo.
    ah/al stationary slices [128, 2, <=128]; bh/bl moving [128, 2, <=256]."""
    terms = ((ah, bh), (ah, bl), (al, bh))
    for i, (a, b) in enumerate(terms):
        nc.tensor.matmul(
            ps, lhsT=a, rhs=b,
            start=(first and i == 0),
            stop=(last and i == 2),
            perf_mode=PM,
        )


def _build_program(mt_in):
    nc = bacc.Bacc(
        "TRN2",
        target_bir_lowering=False,
        debug=False,
        enable_asserts=False,
        num_devices=8,
    )

    x8h = nc.dram_tensor("x8h", [128, CT, N], F8, kind="ExternalInput")
    x8l = nc.dram_tensor("x8l", [128, CT, N], F8, kind="ExternalInput")
    s8h = nc.dram_tensor("s8h", [128, CT, M], F8, kind="ExternalInput")
    s8l = nc.dram_tensor("s8l", [128, CT, M], F8, kind="ExternalInput")
    wq8h = nc.dram_tensor("wq8h", [128, CT, C], F8, kind="ExternalInput")
    wq8l = nc.dram_tensor("wq8l", [128, CT, C], F8, kind="ExternalInput")
    # wk is d-tile-major so per-dt DMA slices are contiguous: the first
    # k-group only needs 1/8th of the weight bytes before starting
    wk8h = nc.dram_tensor("wk8h", [128, CT, CT, 128], F8, kind="ExternalInput")
    wk8l = nc.dram_tensor("wk8l", [128, CT, CT, 128], F8, kind="ExternalInput")
    wv8h = nc.dram_tensor("wv8h", [128, CT, C], F8, kind="ExternalInput")
    wv8l = nc.dram_tensor("wv8l", [128, CT, C], F8, kind="ExternalInput")
    pw8h = nc.dram_tensor("pw8h", [128, CT, C], F8, kind="ExternalInput")
    pw8l = nc.dram_tensor("pw8l", [128, CT, C], F8, kind="ExternalInput")
    maskf = nc.dram_tensor("maskf", [128, max(mt_in, 1)], F32,
                           kind="ExternalInput")
    biasb = nc.dram_tensor("biasb", [128, C], F32, kind="ExternalInput")
    out = nc.dram_tensor("out", [N, C], F32, kind="ExternalOutput")

    with tile.TileContext(nc, pool_alloc_mode="queue") as tc:
        _trace_kernel(tc, mt_in, x8h, x8l, s8h, s8l, wq8h, wq8l, wk8h, wk8l,
                      wv8h, wv8l, pw8h, pw8l, maskf, biasb, out)
    nc.compile()
    return nc


def _trace_kernel(tc, mt_in, x8h, x8l, s8h, s8l, wq8h, wq8l, wk8h, wk8l,
                  wv8h, wv8l, pw8h, pw8l, maskf, biasb, out):
    nc = tc.nc
    from contextlib import ExitStack

    MT = M // 128

    with ExitStack() as ctx:
        persist = ctx.enter_context(tc.tile_pool(name="persist", bufs=1))
        maskt = persist.tile([128, max(mt_in, 1)], F32, tag="maskt")
        ones = persist.tile([128, 1], BF16, tag="ones")
        nc.vector.memset(ones[:], 1.0)
        bias = persist.tile([128, C], F32, tag="bias")

        # persistent activation-derived tensors (fp8 hi/lo pairs)
        kth = persist.tile([128, CT, M], F8, tag="kth")
        ktl = persist.tile([128, CT, M], F8, tag="ktl")
        v = persist.tile([128, max(mt_in, 1), C], BF16, tag="v")
        pwh = persist.tile([128, CT, C], F8, tag="pwh")
        pwl = persist.tile([128, CT, C], F8, tag="pwl")

        # ------------- build phase: kT and v (fp8 DoubleRow 3-term) -------
        with (
            tc.tile_pool(name="w8", bufs=1) as w8p,
            tc.tile_pool(name="sp8", bufs=1) as sp8,
            tc.tile_pool(name="bld", bufs=6, space="PSUM") as bld,
        ):
            wkh = w8p.tile([128, CT, CT, 128], F8, tag="wkh")
            wkl = w8p.tile([128, CT, CT, 128], F8, tag="wkl")
            wvh = w8p.tile([128, CT, C], F8, tag="wvh")
            wvl = w8p.tile([128, CT, C], F8, tag="wvl")
            sph = sp8.tile([128, CT, M], F8, tag="sph")
            spl = sp8.tile([128, CT, M], F8, tag="spl")
            # big coalesced DMAs (>=512B innermost runs), ordered so the first
            # k-group's operands (wk d-tile 0 + first support chunk) land first
            nc.sync.dma_start(wkh[:, 0, :, :], wk8h[:, 0, :, :])
            nc.sync.dma_start(wkl[:, 0, :, :], wk8l[:, 0, :, :])
            nc.sync.dma_start(sph[:, :, 0:512], s8h[:, :, 0:512])
            nc.sync.dma_start(spl[:, :, 0:512], s8l[:, :, 0:512])
            for dt in range(1, CT):
                nc.sync.dma_start(wkh[:, dt, :, :], wk8h[:, dt, :, :])
                nc.sync.dma_start(wkl[:, dt, :, :], wk8l[:, dt, :, :])
            nc.sync.dma_start(sph[:, :, 512:1024], s8h[:, :, 512:1024])
            nc.sync.dma_start(spl[:, :, 512:1024], s8l[:, :, 512:1024])
            nc.sync.dma_start(wvh[:], wv8h[:])
            nc.sync.dma_start(wvl[:], wv8l[:])
            nc.sync.dma_start(maskt[:], maskf[:])
            nc.sync.dma_start(bias[:], biasb[:])
            for mc in range(2, M // 512):
                sl = slice(mc * 512, (mc + 1) * 512)
                nc.sync.dma_start(sph[:, :, sl], s8h[:, :, sl])
                nc.sync.dma_start(spl[:, :, sl], s8l[:, :, sl])

            def k_group(mc, dt):
                sl = slice(mc * 512, (mc + 1) * 512)
                ps = bld.tile([128, 512], F32, tag="bld")
                for pr in range(CT // 2):
                    _dr3(nc, ps[:],
                         wkh[:, dt, 2 * pr:2 * pr + 2, :],
                         wkl[:, dt, 2 * pr:2 * pr + 2, :],
                         sph[:, 2 * pr:2 * pr + 2, sl],
                         spl[:, 2 * pr:2 * pr + 2, sl],
                         pr == 0, pr == CT // 2 - 1)
                nc.scalar.copy(kth[:, dt, sl], ps[:])
                nc.vector.tensor_sub(
                    ktl[:, dt, sl], ps[:], kth[:, dt, sl])

            for mc in range(M // 512):
                for dt in range(CT):
                    k_group(mc, dt)
            for mt in range(mt_in):
                for dc in range(C // 512):
                    ps = bld.tile([128, 512], F32, tag="bld")
                    dsl = slice(dc * 512, (dc + 1) * 512)
                    for pr in range(CT // 2):
                        _dr3(nc, ps[:],
                             sph[:, 2 * pr:2 * pr + 2,
                                 mt * 128:(mt + 1) * 128],
                             spl[:, 2 * pr:2 * pr + 2,
                                 mt * 128:(mt + 1) * 128],
                             wvh[:, 2 * pr:2 * pr + 2, dsl],
                             wvl[:, 2 * pr:2 * pr + 2, dsl],
                             pr == 0, pr == CT // 2 - 1)
                    nc.vector.tensor_scalar_mul(
                        v[:, mt, dsl], ps[:], maskt[:, mt:mt + 1])

        # ------------- attention + interleaved projection ------------------
        with (
            tc.tile_pool(name="wq8", bufs=1) as wq8p,
            tc.tile_pool(name="x8", bufs=2) as x8p,
            tc.tile_pool(name="qt", bufs=2) as qtp,
            tc.tile_pool(name="pt", bufs=17) as ptp,
            tc.tile_pool(name="ob", bufs=4) as obp,
            tc.tile_pool(name="fo", bufs=5) as fop,
            tc.tile_pool(name="st", bufs=4) as stp,
            tc.tile_pool(name="sqs", bufs=4, space="PSUM") as sqs,
            tc.tile_pool(name="ops", bufs=2, space="PSUM") as ops,
            tc.tile_pool(name="djs", bufs=2, space="PSUM") as djs,
        ):
            wqh = wq8p.tile([128, CT, C], F8, tag="wqh")
            wql = wq8p.tile([128, CT, C], F8, tag="wql")
            nc.sync.dma_start(wqh[:], wq8h[:])
            nc.sync.dma_start(wql[:], wq8l[:])
            nc.sync.dma_start(pwh[:], pw8h[:])
            nc.sync.dma_start(pwl[:], pw8l[:])

            x8 = [None, None]

            def load_x(c):
                xh = x8p.tile([128, CT, NF], F8, tag="xh")
                xl = x8p.tile([128, CT, NF], F8, tag="xl")
                nsl = slice(c * NF, (c + 1) * NF)
                nc.sync.dma_start(xh[:], x8h[:, :, nsl])
                nc.sync.dma_start(xl[:], x8l[:, :, nsl])
                return xh, xl

            x8[0] = load_x(0)
            x8[1] = load_x(1)

            out_v = out[:].rearrange("(t two) d -> two t d", two=2)
            o_half = [None, None]
            pending = []

            def flush_normalize():
                # deferred DVE normalize of the previous n-tile: emitted after
                # the next chunk's q-phase so DVE's in-order queue can't block
                # the q psum recycling at the chunk boundary
                while pending:
                    o_ps, dn, ntile = pending.pop(0)
                    recip = stp.tile([128, 1], F32, tag="recip")
                    nc.vector.reciprocal(recip[:], dn[:, 0:1])
                    ohh, ohl = o_half[(ntile // 8) % 2]
                    j = ntile % 8
                    for dh in range(2):
                        dsl = slice(dh * 512, (dh + 1) * 512)
                        nc.scalar.activation(
                            ohh[:, j, dsl], o_ps[dh][:], AF.Copy,
                            scale=recip[:])
                        nc.vector.scalar_tensor_tensor(
                            ohl[:, j, dsl], o_ps[dh][:], recip[:],
                            ohh[:, j, dsl],
                            op0=mybir.AluOpType.mult,
                            op1=mybir.AluOpType.subtract)

            for c in range(NCH):
                xh, xl = x8[c % 2]
                # qT for this chunk (fp8 DoubleRow 3-term), split hi/lo
                qth = qtp.tile([128, CT, NF], F8, tag="qth")
                qtl = qtp.tile([128, CT, NF], F8, tag="qtl")
                for dt in range(CT):
                    ps = sqs.tile([128, 512], F32, tag="sqs")
                    for pr in range(CT // 2):
                        _dr3(nc, ps[:],
                             wqh[:, 2 * pr:2 * pr + 2,
                                 dt * 128:(dt + 1) * 128],
                             wql[:, 2 * pr:2 * pr + 2,
                                 dt * 128:(dt + 1) * 128],
                             xh[:, 2 * pr:2 * pr + 2, :],
                             xl[:, 2 * pr:2 * pr + 2, :],
                             pr == 0, pr == CT // 2 - 1)
                    nc.scalar.copy(qth[:, dt, :], ps[:])
                    nc.vector.tensor_sub(
                        qtl[:, dt, :], ps[:], qth[:, dt, :])
                if c + 2 < NCH:
                    x8[c % 2] = load_x(c + 2)
                flush_normalize()

                # sT per (m-tile, n-half) fp8 DoubleRow 3-term, exp -> pT bf16
                pts = []
                for mt in range(MT):
                    pt = ptp.tile([128, NF], BF16, tag="pt")
                    ps = sqs.tile([128, 512], F32, tag="sqs")
                    for pr in range(CT // 2):
                        _dr3(nc, ps[:],
                             kth[:, 2 * pr:2 * pr + 2,
                                 mt * 128:(mt + 1) * 128],
                             ktl[:, 2 * pr:2 * pr + 2,
                                 mt * 128:(mt + 1) * 128],
                             qth[:, 2 * pr:2 * pr + 2, :],
                             qtl[:, 2 * pr:2 * pr + 2, :],
                             pr == 0, pr == CT // 2 - 1)
                    nc.scalar.activation(
                        pt[:], ps[:], AF.Exp, scale=EXPSCALE)
                    pts.append(pt)

                # p@v + denominator + normalize, per n-tile of 128
                for nt in range(NF // 128):
                    ntile = c * (NF // 128) + nt
                    nsl = slice(nt * 128, (nt + 1) * 128)
                    o_ps = []
                    for dh in range(2):
                        ps = ops.tile([128, 512], F32, tag="ops")
                        for mt in range(mt_in):
                            nc.tensor.matmul(
                                ps[:],
                                lhsT=pts[mt][:, nsl],
                                rhs=v[:, mt, dh * 512:(dh + 1) * 512],
                                start=(mt == 0),
                                stop=(mt == mt_in - 1),
                            )
                        o_ps.append(ps)
                    dn = djs.tile([128, 512], F32, tag="djs")
                    for mt in range(MT):
                        nc.tensor.matmul(
                            dn[:, 0:1],
                            lhsT=pts[mt][:, nsl],
                            rhs=ones[:],
                            start=(mt == 0),
                            stop=(mt == MT - 1),
                        )
                    if ntile % 8 == 0:
                        ohh_new = obp.tile([128, 8, C], F8, tag="obh")
                        ohl_new = obp.tile([128, 8, C], F8, tag="obl")
                        o_half[(ntile // 8) % 2] = (ohh_new, ohl_new)
                    if c % 2 == 0 and nt == NF // 128 - 1:
                        pending.append((o_ps, dn, ntile))
                        continue
                    recip = stp.tile([128, 1], F32, tag="recip")
                    nc.vector.reciprocal(recip[:], dn[:, 0:1])
                    ohh, ohl = o_half[(ntile // 8) % 2]
                    j = ntile % 8
                    for dh in range(2):
                        dsl = slice(dh * 512, (dh + 1) * 512)
                        nc.scalar.activation(
                            ohh[:, j, dsl], o_ps[dh][:], AF.Copy,
                            scale=recip[:])
                        nc.vector.scalar_tensor_tensor(
                            ohl[:, j, dsl], o_ps[dh][:], recip[:],
                            ohh[:, j, dsl],
                            op0=mybir.AluOpType.mult,
                            op1=mybir.AluOpType.subtract)

                # after each half: projection with the swapaxes fold
                # (fp8 DoubleRow 3-term; pw scaled x32 on host, undone in the
                # bias add)
                if c % 2 == 1:
                    h = c // 2
                    ohh, ohl = o_half[h % 2]
                    for tt in range(CT):
                        for dc in range(C // 512):
                            ps = djs.tile([128, 512], F32, tag="djs")
                            dsl = slice(dc * 512, (dc + 1) * 512)
                            for pr in range(CT // 2):
                                _dr3(nc, ps[:],
                                     ohh[:, 2 * pr:2 * pr + 2,
                                         tt * 128:(tt + 1) * 128],
                                     ohl[:, 2 * pr:2 * pr + 2,
                                         tt * 128:(tt + 1) * 128],
                                     pwh[:, 2 * pr:2 * pr + 2, dsl],
                                     pwl[:, 2 * pr:2 * pr + 2, dsl],
                                     pr == 0, pr == CT // 2 - 1)
                            f_sb = fop.tile([128, 512], F32, tag="fo")
                            nc.vector.scalar_tensor_tensor(
                                f_sb[:], ps[:], 1.0 / WS,
                                bias[:, dsl],
                                op0=mybir.AluOpType.mult,
                                op1=mybir.AluOpType.add)
                            nc.sync.dma_start(
                                out_v[h, tt * 128:(tt + 1) * 128, dsl],
                                f_sb[:],
                            )


def _prep_layout(a):
    # a [rows(c), cols] -> [128, CT, cols] with c = ct*128 + p
    cols = a.shape[1]
    return np.ascontiguousarray(
        a.reshape(CT, 128, cols).transpose(1, 0, 2))


def _prep_layout_dt(a):
    # a [rows(c), cols(d)] -> [128, CT(dt), CT(ct), 128] with c = ct*128 + p,
    # d = dt*128 + dd (d-tile-major for per-dt contiguous DMA slices)
    return np.ascontiguousarray(
        a.reshape(CT, 128, CT, 128).transpose(1, 2, 0, 3))


def _hl(a):
    hi = a.astype(F8NP)
    lo = (a - hi.astype(np.float32)).astype(F8NP)
    return np.ascontiguousarray(hi), np.ascontiguousarray(lo)


def prep_in_maps(x, support, attn_mask, qkv_w, proj_w, proj_b):
    x = np.asarray(x, dtype=np.float32)
    support = np.asarray(support, dtype=np.float32)
    attn_mask = np.asarray(attn_mask)
    qkv_w = np.asarray(qkv_w, dtype=np.float32)
    proj_w = np.asarray(proj_w, dtype=np.float32)
    proj_b = np.asarray(proj_b, dtype=np.float32)

    mask = (attn_mask != 0)
    perm = np.argsort(~mask, kind="stable")
    m1 = int(mask.sum())
    mt_in = max((m1 + 127) // 128, 1)
    mask_perm = mask[perm].astype(np.float32)

    wq = qkv_w[:C] * WS
    wk = qkv_w[C:2 * C] * WS
    wv = qkv_w[2 * C:] * WS
    wq8h, wq8l = _hl(_prep_layout(wq.T))
    wk8h, wk8l = _hl(_prep_layout_dt(wk.T))
    wv8h, wv8l = _hl(_prep_layout(wv.T))
    pw8h, pw8l = _hl(_prep_layout(proj_w.T * WS))
    maskf = np.ascontiguousarray(
        (mask_perm[:mt_in * 128] / WS).reshape(mt_in, 128).T)
    biasb = np.ascontiguousarray(
        np.broadcast_to(proj_b, (128, C)).astype(np.float32))

    in_maps = []
    for b in range(B):
        x8h, x8l = _hl(_prep_layout(x[b].T))
        s8h, s8l = _hl(_prep_layout(support[b][perm].T))
        in_maps.append({
            "x8h": x8h, "x8l": x8l, "s8h": s8h, "s8l": s8l,
            "wq8h": wq8h, "wq8l": wq8l, "wk8h": wk8h, "wk8l": wk8l,
            "wv8h": wv8h, "wv8l": wv8l, "pw8h": pw8h, "pw8l": pw8l,
            "maskf": maskf, "biasb": biasb,
        })
    return in_maps, mt_in


def kernel(x, support, attn_mask, qkv_w, proj_w, proj_b):
    in_maps, mt_in = prep_in_maps(x, support, attn_mask, qkv_w, proj_w,
                                  proj_b)
    if mt_in not in _CACHE:
        _CACHE[mt_in] = _build_program(mt_in)
    nc = _CACHE[mt_in]
    _CACHE["nc"] = nc

    res = run_bass_kernel_spmd(nc, in_maps, core_ids=list(range(B)))
    return np.stack([res.results[b]["out"] for b in range(B)], axis=0)
